# revision 1
# baseline (speedup 1.0000x reference)
"""Trainium2 Bass kernel for nn_MemoryEfficientCrossAttention (WPR-pruned attention).

Self-contained: hardcodes shapes/sharding. The harness calls kernel(**inputs).

Pipeline (4 SPMD launches on 8 NeuronCores, host does only data movement):
  P1 proj:  core c: tokens [c*512,(c+1)*512) of flattened x -> qT,kT (fp16 hi/lo
            pairs, [1152,512] each) and v ([512,1152] f32).  fp32 matmuls.
  P2 map:   core (b,j): query rows j*512..+512 of batch b, all 16 heads ->
            attn row-chunk [512,2048] f32 (mean over heads of softmax).
            Logits via fp16 hi/lo pair matmuls (3 products, exact to ~2^-22),
            exp without max-subtraction (logits bounded ~|3|), fused
            normalize-accumulate over heads.
  P3 sel:   core (b,j): full attn[b] -> 5-step power iteration (fp32 matvec),
            strict rank -> keep mask; argmax source rows for column quarter.
  P4 attnr: core (b,j): retained-token quarter -> masked attention over
            retained keys (S^T fp16-pairs, exp with -1e30 mask bias, PV fp32
            with fused ones-column rowsums), /rowsum, @Wo + bo -> finalT chunk.
  Host: scatter retained rows, recovery copy final[prune] = final[src[prune]].
"""

import numpy as np

import concourse.bass as bass
import concourse.mybir as mybir
import concourse.tile as tile
from concourse.bass_utils import run_bass_kernel_spmd

F32 = mybir.dt.float32
F16 = mybir.dt.float16
AF = mybir.ActivationFunctionType

B, N, QD, H, D = 2, 2048, 1152, 16, 72
INNER = H * D
N_KEEP = 819
SCALE = np.float32(D ** -0.5)
P = 128
NC = 8
CHUNK = 512          # tokens per core in P1/P2
KT = QD // P         # 9 k-tiles of the 1152 contraction
NQ4 = 205            # max retained tokens per quarter-core in P4 (819 -> 205,205,205,204)
NQP = 208            # padded

_CORE_IDS = list(range(NC))


def split_waits(nc, maxw=1):
    """This toolchain's walrus accepts only one sync-wait per instruction;
    move excess waits onto preceding same-engine EventSemaphore nops."""
    n_new = 0
    for f in nc.m.functions:
        for blk in f.blocks:
            out = []
            changed = False
            for inst in blk.instructions:
                si = inst.sync_info
                if si is not None and si.on_wait is not None and len(si.on_wait) > maxw:
                    waits = list(si.on_wait)
                    for w in waits[:-maxw]:
                        es = mybir.InstEventSemaphore(
                            name=f"Wsplit{n_new}", ins=[], outs=[])
                        es.engine = inst.engine
                        es.sync_info = mybir.SyncInfo(on_wait=[w], on_update=[])
                        out.append(es)
                        n_new += 1
                    si.on_wait = waits[-maxw:]
                    changed = True
                out.append(inst)
            if changed:
                blk.instructions = out
    return nc


# --------------------------------------------------------------------------
# P2: attention map.  per core (b, j): query rows [j*512,(j+1)*512) of batch b
# -> attn row-chunk [512, 2048] f32 = mean over heads of row-softmax.
# --------------------------------------------------------------------------

def build_map():
    nc = bass.Bass("TRN2", target_bir_lowering=False, debug=False, num_devices=NC)
    qh = nc.dram_tensor("qh", [INNER, CHUNK], F16, kind="ExternalInput").ap()
    ql = nc.dram_tensor("ql", [INNER, CHUNK], F16, kind="ExternalInput").ap()
    kh = nc.dram_tensor("kh", [INNER, N], F16, kind="ExternalInput").ap()
    kl = nc.dram_tensor("kl", [INNER, N], F16, kind="ExternalInput").ap()
    attn = nc.dram_tensor("attn", [CHUNK, N], F32, kind="ExternalOutput").ap()

    NSEG = N // 512  # 4 column segments per row
    with tile.TileContext(nc) as tc:
        with tc.tile_pool(name="kp", bufs=1) as kp, \
             tc.tile_pool(name="qp", bufs=3) as qp, \
             tc.tile_pool(name="ep", bufs=3) as ep, \
             tc.tile_pool(name="ap", bufs=2) as ap_, \
             tc.tile_pool(name="sp", bufs=4) as sp, \
             tc.tile_pool(name="ps", bufs=2, space="PSUM") as ps:
            # resident k pairs, per head [72, 2048]; q pairs [72, 16, 512]
            kht = kp.tile([72, H, N], F16)
            klt = kp.tile([72, H, N], F16)
            nc.sync.dma_start(kht[:], kh.rearrange("(h d) m -> d h m", d=D))
            nc.sync.dma_start(klt[:], kl.rearrange("(h d) m -> d h m", d=D))
            qha = kp.tile([72, H, CHUNK], F16)
            qla = kp.tile([72, H, CHUNK], F16)
            nc.sync.dma_start(qha[:], qh.rearrange("(h d) m -> d h m", d=D))
            nc.sync.dma_start(qla[:], ql.rearrange("(h d) m -> d h m", d=D))
            for nqt in range(CHUNK // P):
                acc = ap_.tile([P, N], F32, tag="acc")
                for h in range(H):
                    qht = qha[:, h, nqt * P:(nqt + 1) * P]
                    qlt = qla[:, h, nqt * P:(nqt + 1) * P]
                    et = ep.tile([P, N], F32, tag="et")
                    rs = sp.tile([P, 1], F32, tag="rs")
                    pt4 = ps.tile([P, N], F32, tag="pt4")
                    for ms in range(NSEG):
                        seg = slice(ms * 512, (ms + 1) * 512)
                        nc.tensor.matmul(pt4[:, seg], qht, kht[:, h, seg],
                                         start=True, stop=False)
                        nc.tensor.matmul(pt4[:, seg], qht, klt[:, h, seg],
                                         start=False, stop=False)
                        nc.tensor.matmul(pt4[:, seg], qlt, kht[:, h, seg],
                                         start=False, stop=True)
                    nc.scalar.activation(et[:], pt4[:], AF.Exp,
                                         scale=float(SCALE), accum_out=rs[:])
                    w = sp.tile([P, 1], F32, tag="w")
                    nc.vector.tensor_scalar_mul(rs[:], rs[:], float(H))
                    nc.vector.reciprocal(w[:], rs[:])
                    if h == 0:
                        nc.vector.tensor_scalar(
                            acc[:], et[:], w[:], scalar2=None,
                            op0=mybir.AluOpType.mult)
                    else:
                        nc.vector.scalar_tensor_tensor(
                            acc[:], et[:], w[:], acc[:],
                            op0=mybir.AluOpType.mult, op1=mybir.AluOpType.add)
                nc.sync.dma_start(attn[nqt * P:(nqt + 1) * P, :], acc[:])
    return split_waits(nc)


def run_map(proj, trace=False):
    in_maps = []
    for c in range(NC):
        b, j = divmod(c, 4)
        sl = slice(j * CHUNK, (j + 1) * CHUNK)
        in_maps.append({
            "qh": np.ascontiguousarray(proj["qhT"][b][:, sl]),
            "ql": np.ascontiguousarray(proj["qlT"][b][:, sl]),
            "kh": proj["khT"][b], "kl": proj["klT"][b],
        })
    res = run_bass_kernel_spmd(build_map(), in_maps, core_ids=_CORE_IDS, trace=trace)
    attn = [np.concatenate([res.results[b * 4 + j]["attn"] for j in range(4)], axis=0)
            for b in range(B)]
    return attn, res


# --------------------------------------------------------------------------
# P3: selection.  per core (b, j): full attn[b] [2048,2048] ->
#   keep mask [2048] (top-819 by 5-step power-iteration importance, strict rank)
#   srcq [512]: for column quarter j, the retained row index with max attention.
#   imp [2048]: importance (diagnostics).
# --------------------------------------------------------------------------

def build_sel():
    from concourse.masks import make_identity
    nc = bass.Bass("TRN2", target_bir_lowering=False, debug=False, num_devices=NC)
    attn = nc.dram_tensor("attn", [N, N], F32, kind="ExternalInput").ap()
    attn_q = nc.dram_tensor("attn_q", [N, CHUNK], F32, kind="ExternalInput").ap()
    keep_o = nc.dram_tensor("keep", [1, N], F32, kind="ExternalOutput").ap()
    imp_o = nc.dram_tensor("imp", [1, N], F32, kind="ExternalOutput").ap()
    srcq_o = nc.dram_tensor("srcq", [1, CHUNK], F32, kind="ExternalOutput").ap()

    NT = N // P  # 16
    BIG = float(1 << 24)   # integer-exact in f32
    with tile.TileContext(nc) as tc:
        with tc.tile_pool(name="Ap", bufs=1) as Ap, \
             tc.tile_pool(name="cp", bufs=1) as cp, \
             tc.tile_pool(name="dp", bufs=2) as dp, \
             tc.tile_pool(name="rp", bufs=1) as rp, \
             tc.tile_pool(name="tp", bufs=4) as tp, \
             tc.tile_pool(name="atp", bufs=2) as atp, \
             tc.tile_pool(name="ps", bufs=2, space="PSUM") as ps, \
             tc.tile_pool(name="ps1", bufs=2, space="PSUM") as ps1:
            At = Ap.tile([P, NT, N], F32)          # attn row-tiles, resident
            nc.sync.dma_start(At[:], attn.rearrange("(kt p) m -> p kt m", p=P))
            ident = cp.tile([P, P], F32)
            make_identity(nc, ident[:])
            ones_col = cp.tile([1, P], F32)
            nc.vector.memset(ones_col[:], 1.0)

            # ---- 5-step power iteration, dist kept column-major [128, 16]
            dist = dp.tile([P, NT], F32, tag="dist")
            nc.vector.memset(dist[:], 1.0 / N)
            for it in range(5):
                ndist = dp.tile([P, NT], F32, tag="dist")
                for mt in range(NT):
                    pd = ps.tile([P, 1], F32, tag="pd")
                    for kt in range(NT):
                        nc.tensor.matmul(pd[:], At[:, kt, mt * P:(mt + 1) * P],
                                         dist[:, kt:kt + 1],
                                         start=(kt == 0), stop=(kt == NT - 1))
                    nc.scalar.copy(ndist[:, mt:mt + 1], pd[:])
                dist = ndist

            # ---- importance row [1, 2048] via PE transposes
            imp_row = rp.tile([1, N], F32)
            for kt in range(NT):
                pr = ps.tile([1, P], F32, tag="pr")
                nc.tensor.transpose(pr[:], dist[:, kt:kt + 1], ident[:])
                nc.scalar.copy(imp_row[:, kt * P:(kt + 1) * P], pr[:])
            nc.sync.dma_start(imp_o[:], imp_row[:])

            # ---- imp broadcast [128, 2048] (ones ⊗ imp_row)
            impb = rp.tile([P, N], F32)
            for ms in range(N // 512):
                pb = ps.tile([P, 512], F32, tag="pb")
                nc.tensor.matmul(pb[:], ones_col[:],
                                 imp_row[:, ms * 512:(ms + 1) * 512],
                                 start=True, stop=True)
                nc.scalar.copy(impb[:, ms * 512:(ms + 1) * 512], pb[:])

            # ---- strict rank + keep mask, column-major then row
            scratch = rp.tile([P, N], F32)
            keep_col = dp.tile([P, NT], F32, tag="keepc")
            for kt in range(NT):
                rank = tp.tile([P, 1], F32, tag="rank")
                nc.vector.tensor_scalar(
                    scratch[:], impb[:], dist[:, kt:kt + 1], scalar2=None,
                    op0=mybir.AluOpType.is_gt)
                nc.scalar.activation(scratch[:], scratch[:], AF.Identity,
                                     accum_out=rank[:])
                nc.vector.tensor_scalar(
                    keep_col[:, kt:kt + 1], rank[:], float(N_KEEP), scalar2=None,
                    op0=mybir.AluOpType.is_lt)
            keep_row = rp.tile([1, N], F32)
            for kt in range(NT):
                pk = ps.tile([1, P], F32, tag="pr")
                nc.tensor.transpose(pk[:], keep_col[:, kt:kt + 1], ident[:])
                nc.scalar.copy(keep_row[:, kt * P:(kt + 1) * P], pk[:])
            nc.sync.dma_start(keep_o[:], keep_row[:])

            # ---- neg bias rows: (keep-1)*BIG -> 0 keep / -BIG pruned
            negb_row = rp.tile([1, N], F32)
            nc.vector.tensor_scalar(
                negb_row[:], keep_row[:], 1.0, scalar2=BIG,
                op0=mybir.AluOpType.subtract, op1=mybir.AluOpType.mult)
            negb = scratch
            iotaB = impb
            for ms in range(N // 512):
                pb2 = ps.tile([P, 512], F32, tag="pb")
                nc.tensor.matmul(pb2[:], ones_col[:],
                                 negb_row[:, ms * 512:(ms + 1) * 512],
                                 start=True, stop=True)
                nc.scalar.copy(negb[:, ms * 512:(ms + 1) * 512], pb2[:])
            nc.gpsimd.iota(iotaB[:], pattern=[[1, N]], base=0,
                           channel_multiplier=0,
                           allow_small_or_imprecise_dtypes=True)
            nc.vector.tensor_scalar_sub(iotaB[:], iotaB[:], BIG)

            # ---- argmax over retained rows for this core's column quarter
            for mt in range(4):
                aq = atp.tile([P, NT, P], F32, tag="aq")
                nc.sync.dma_start(
                    aq[:], attn_q.rearrange("(kt p) m -> p kt m",
                                            p=P)[:, :, mt * P:(mt + 1) * P])
                at_t = atp.tile([P, N], F32, tag="att")
                for kt in range(NT):
                    pt2 = ps1.tile([P, P], F32, tag="pt2")
                    nc.tensor.transpose(pt2[:], aq[:, kt], ident[:])
                    nc.scalar.copy(at_t[:, kt * P:(kt + 1) * P], pt2[:])
                nc.vector.tensor_add(at_t[:], at_t[:], negb[:])
                mx = tp.tile([P, 1], F32, tag="mx")
                nc.vector.reduce_max(mx[:], at_t[:], axis=mybir.AxisListType.X)
                nc.vector.tensor_scalar(
                    at_t[:], at_t[:], mx[:], scalar2=None,
                    op0=mybir.AluOpType.is_equal)
                nc.vector.tensor_mul(at_t[:], at_t[:], iotaB[:])
                idx = tp.tile([P, 1], F32, tag="idx")
                nc.vector.tensor_reduce(idx[:], at_t[:],
                                        axis=mybir.AxisListType.X,
                                        op=mybir.AluOpType.min)
                nc.vector.tensor_scalar_add(idx[:], idx[:], BIG)
                psr = ps.tile([1, P], F32, tag="pr")
                nc.tensor.transpose(psr[:], idx[:], ident[:])
                src_row = tp.tile([1, P], F32, tag="srow")
                nc.scalar.copy(src_row[:], psr[:])
                nc.sync.dma_start(srcq_o[0:1, mt * P:(mt + 1) * P], src_row[:])
    return split_waits(nc)


def run_sel(attn, trace=False):
    in_maps = []
    for c in range(NC):
        b, j = divmod(c, 4)
        in_maps.append({
            "attn": attn[b],
            "attn_q": np.ascontiguousarray(attn[b][:, j * CHUNK:(j + 1) * CHUNK]),
        })
    res = run_bass_kernel_spmd(build_sel(), in_maps, core_ids=_CORE_IDS, trace=trace)
    out = []
    for b in range(B):
        keep = res.results[b * 4]["keep"][0]
        imp = res.results[b * 4]["imp"][0]
        src = np.concatenate(
            [res.results[b * 4 + j]["srcq"][0] for j in range(4)])
        out.append({"keep": keep, "imp": imp, "src": src.astype(np.int64)})
    return out, res


# --------------------------------------------------------------------------
# P4: retained attention + output projection.
# per core (b, j): ~205 retained tokens (host-gathered q columns, padded to
# NQP) -> finT [1152, NQP] = (masked-softmax(qk) @ v / rowsum) @ Wo + bo,
# transposed.  S^T via fp16 pairs; exp with per-partition keep bias; PV fp32
# with a fused ones-column giving rowsums; per-head normalize; Wo projection.
# --------------------------------------------------------------------------

MP = 896             # padded retained-key count (819 -> 7 tiles of 128)


def build_attnr():
    nc = bass.Bass("TRN2", target_bir_lowering=False, debug=False, num_devices=NC)
    qhs = nc.dram_tensor("qhs", [INNER, NQP], F16, kind="ExternalInput").ap()
    qls = nc.dram_tensor("qls", [INNER, NQP], F16, kind="ExternalInput").ap()
    kh = nc.dram_tensor("kh", [INNER, MP], F16, kind="ExternalInput").ap()
    kl = nc.dram_tensor("kl", [INNER, MP], F16, kind="ExternalInput").ap()
    v97 = nc.dram_tensor("v97", [MP // P, H, P, 97], F32,
                         kind="ExternalInput").ap()  # v cols 0..71, ones col 96
    keepc = nc.dram_tensor("keepc", [P, MP // P], F32, kind="ExternalInput").ap()
    wo = nc.dram_tensor("wo", [INNER, INNER], F32, kind="ExternalInput").ap()
    boc = nc.dram_tensor("boc", [P, KT], F32, kind="ExternalInput").ap()
    finT = nc.dram_tensor("finT", [INNER, NQP], F32, kind="ExternalOutput").ap()

    NT = MP // P
    BIGEXP = 30000.0
    with tile.TileContext(nc) as tc:
        with tc.tile_pool(name="kp", bufs=1) as kp, \
             tc.tile_pool(name="qp", bufs=1) as qp, \
             tc.tile_pool(name="vp", bufs=4) as vp, \
             tc.tile_pool(name="ep", bufs=4) as ep, \
             tc.tile_pool(name="np_", bufs=2) as np_, \
             tc.tile_pool(name="op", bufs=1) as op, \
             tc.tile_pool(name="wp", bufs=3) as wp, \
             tc.tile_pool(name="cp", bufs=1) as cp, \
             tc.tile_pool(name="psS", bufs=4, space="PSUM") as psS, \
             tc.tile_pool(name="psO", bufs=2, space="PSUM") as psO, \
             tc.tile_pool(name="psB", bufs=1, space="PSUM") as psB, \
             tc.tile_pool(name="psF", bufs=1, space="PSUM") as psF:
            kht = kp.tile([72, H, MP], F16)
            klt = kp.tile([72, H, MP], F16)
            nc.sync.dma_start(kht[:], kh.rearrange("(h d) m -> d h m", d=D))
            nc.sync.dma_start(klt[:], kl.rearrange("(h d) m -> d h m", d=D))
            qht = qp.tile([72, H, NQP], F16)
            qlt = qp.tile([72, H, NQP], F16)
            nc.sync.dma_start(qht[:], qhs.rearrange("(h d) m -> d h m", d=D))
            nc.sync.dma_start(qlt[:], qls.rearrange("(h d) m -> d h m", d=D))
            vr = kp.tile([P, MP // P, H, 97], F32)
            nc.sync.dma_start(vr[:], v97.rearrange("mc h p c -> p mc h c"))
            wot = kp.tile([P, KT, INNER], F32)
            nc.sync.dma_start(wot[:], wo.rearrange("(kt p) m -> p kt m", p=P))
            maskb = cp.tile([P, NT], F32)
            nc.sync.dma_start(maskb[:], keepc[:])
            nc.vector.tensor_scalar(
                maskb[:], maskb[:], 1.0, scalar2=BIGEXP,
                op0=mybir.AluOpType.subtract, op1=mybir.AluOpType.mult)
            ones72 = cp.tile([1, 72], F32)
            nc.vector.memset(ones72[:], 1.0)
            bo_sb = cp.tile([P, KT], F32)
            nc.sync.dma_start(bo_sb[:], boc[:])

            ON = op.tile([P, KT, NQP], F32)  # normalized PV output, hd-stacked
            for h in range(H):
                po = psO.tile([97, NQP], F32, tag="po")
                for mc in range(NT):
                    pss = psS.tile([P, NQP], F32, tag="pss")
                    ksl = slice(mc * P, (mc + 1) * P)
                    nc.tensor.matmul(pss[:], kht[:, h, ksl], qht[:, h],
                                     start=True, stop=False)
                    nc.tensor.matmul(pss[:], kht[:, h, ksl], qlt[:, h],
                                     start=False, stop=False)
                    nc.tensor.matmul(pss[:], klt[:, h, ksl], qht[:, h],
                                     start=False, stop=True)
                    et = ep.tile([P, NQP], F32, tag="et")
                    nc.scalar.activation(et[:], pss[:], AF.Exp,
                                         scale=float(SCALE),
                                         bias=maskb[:, mc:mc + 1])
                    nc.tensor.matmul(po[:], vr[:, mc, h], et[:],
                                     start=(mc == 0), stop=(mc == NT - 1))
                # normalize: rows 0..71 / row 96 (the ones-column sums)
                inv = np_.tile([1, NQP], F32, tag="inv")
                nc.vector.reciprocal(inv[:], po[96:97, :])
                pb = psB.tile([72, NQP], F32, tag="pb")
                nc.tensor.matmul(pb[:], ones72[:], inv[:], start=True, stop=True)
                binv = np_.tile([72, NQP], F32, tag="binv")
                nc.scalar.copy(binv[:], pb[:])
                onv = np_.tile([72, NQP], F32, tag="onv")
                nc.vector.tensor_mul(onv[:], po[0:D, :], binv[:])
                g0 = h * D
                t0, p0 = divmod(g0, P)
                n0 = min(D, P - p0)
                nc.sync.dma_start(ON[p0:p0 + n0, t0, :], onv[0:n0, :])
                if n0 < D:
                    nc.sync.dma_start(ON[0:D - n0, t0 + 1, :], onv[n0:D, :])

            # final^T = Wo^T @ ON + bo
            foa = kp.tile([P, KT, NQP], F32)
            for mt in range(KT):
                pf = psF.tile([P, NQP], F32, tag="pf")
                for kt in range(KT):
                    nc.tensor.matmul(pf[:], wot[:, kt, mt * P:(mt + 1) * P],
                                     ON[:, kt, :],
                                     start=(kt == 0), stop=(kt == KT - 1))
                nc.scalar.activation(foa[:, mt], pf[:], AF.Identity,
                                     bias=bo_sb[:, mt:mt + 1])
            nc.sync.dma_start(finT.rearrange("(mt p) m -> p mt m", p=P), foa[:])
    return split_waits(nc)


def run_attnr(proj, sel, Wo, bo, trace=False):
    in_maps = []
    meta = []
    boc = np.ascontiguousarray(bo.reshape(KT, P).T.astype(np.float32))
    for c in range(NC):
        b, j = divmod(c, 4)
        keep = sel[b]["keep"]
        idx = np.nonzero(keep > 0.5)[0]
        bounds = np.linspace(0, len(idx), 5).astype(int)
        my = idx[bounds[j]:bounds[j + 1]]
        meta.append(my)
        qhs = np.zeros((INNER, NQP), np.float16)
        qls = np.zeros((INNER, NQP), np.float16)
        qhs[:, :len(my)] = proj["qhT"][b][:, my]
        qls[:, :len(my)] = proj["qlT"][b][:, my]
        khs = np.zeros((INNER, MP), np.float16)
        kls = np.zeros((INNER, MP), np.float16)
        khs[:, :len(idx)] = proj["khT"][b][:, idx]
        kls[:, :len(idx)] = proj["klT"][b][:, idx]
        vsel = np.zeros((MP, INNER), np.float32)
        vsel[:len(idx)] = proj["v"][b][idx]               # [MP, INNER]
        v97 = np.zeros((MP // P, H, P, 97), np.float32)
        v97[..., :D] = (vsel.reshape(MP // P, P, H, D)).transpose(0, 2, 1, 3)
        v97[..., 96] = 1.0
        keepp = np.zeros(MP, np.float32)
        keepp[:len(idx)] = 1.0
        in_maps.append({
            "qhs": qhs, "qls": qls,
            "kh": khs, "kl": kls,
            "v97": np.ascontiguousarray(v97),
            "keepc": np.ascontiguousarray(
                keepp.reshape(MP // P, P).T.astype(np.float32)),
            "wo": Wo, "boc": boc,
        })
    res = run_bass_kernel_spmd(build_attnr(), in_maps, core_ids=_CORE_IDS, trace=trace)
    out = np.zeros((B, N, INNER), np.float32)
    for c in range(NC):
        b = c // 4
        my = meta[c]
        out[b][my] = res.results[c]["finT"][:, :len(my)].T
    # recovery: pruned tokens copy their most-attending retained token's row
    for b in range(B):
        keep = sel[b]["keep"] > 0.5
        prune = np.nonzero(~keep)[0]
        out[b][prune] = out[b][sel[b]["src"][prune]]
    return out, res


def kernel(x, Wq, Wk, Wv, Wo, bo):
    proj, _ = run_proj(np.asarray(x, np.float32), np.asarray(Wq, np.float32),
                       np.asarray(Wk, np.float32), np.asarray(Wv, np.float32))
    attn, _ = run_map(proj)
    sel, _ = run_sel(attn)
    out, _ = run_attnr(proj, sel, np.asarray(Wo, np.float32),
                       np.asarray(bo, np.float32))
    return out


# --------------------------------------------------------------------------
# P1: projections.  per core: xT_chunk [1152, 512] -> qT/kT pairs, v.
# --------------------------------------------------------------------------

def build_proj():
    nc = bass.Bass("TRN2", target_bir_lowering=False, debug=False, num_devices=NC)
    xT = nc.dram_tensor("xT", [QD, CHUNK], F32, kind="ExternalInput").ap()
    wq = nc.dram_tensor("wq", [QD, INNER], F32, kind="ExternalInput").ap()
    wk = nc.dram_tensor("wk", [QD, INNER], F32, kind="ExternalInput").ap()
    wv = nc.dram_tensor("wv", [QD, INNER], F32, kind="ExternalInput").ap()
    qhT = nc.dram_tensor("qhT", [INNER, CHUNK], F16, kind="ExternalOutput").ap()
    qlT = nc.dram_tensor("qlT", [INNER, CHUNK], F16, kind="ExternalOutput").ap()
    khT = nc.dram_tensor("khT", [INNER, CHUNK], F16, kind="ExternalOutput").ap()
    klT = nc.dram_tensor("klT", [INNER, CHUNK], F16, kind="ExternalOutput").ap()
    vout = nc.dram_tensor("v", [CHUNK, INNER], F32, kind="ExternalOutput").ap()

    with tile.TileContext(nc) as tc:
        with tc.tile_pool(name="xp", bufs=1) as xp, \
             tc.tile_pool(name="sbp", bufs=2) as sbp, \
             tc.tile_pool(name="wp", bufs=2) as wp, \
             tc.tile_pool(name="op", bufs=3) as op, \
             tc.tile_pool(name="vp", bufs=1) as vp, \
             tc.tile_pool(name="ps", bufs=4, space="PSUM") as ps:
            # x pairs resident; f32 staged through sub-band tiles
            xh = xp.tile([P, KT, CHUNK], F16)
            xl = xp.tile([P, KT, CHUNK], F16)
            for sub in range(3):
                xt = sbp.tile([P, 3, CHUNK], F32, tag="xt")
                ksl = slice(sub * 3, sub * 3 + 3)
                nc.sync.dma_start(
                    xt[:], xT.rearrange("(kc p) m -> p kc m", p=P)[:, ksl])
                nc.scalar.copy(xh[:, ksl], xt[:])
                nc.vector.tensor_sub(xl[:, ksl], xt[:], xh[:, ksl])

            def w_pairs(w_ap):
                wh = wp.tile([P, KT, INNER], F16, tag="wh")
                wl = wp.tile([P, KT, INNER], F16, tag="wl")
                for sub in range(3):
                    wt = sbp.tile([P, 3, INNER], F32, tag="wt")
                    ksl = slice(sub * 3, sub * 3 + 3)
                    nc.sync.dma_start(
                        wt[:], w_ap.rearrange("(kc p) m -> p kc m", p=P)[:, ksl])
                    nc.scalar.copy(wh[:, ksl], wt[:])
                    nc.vector.tensor_sub(wl[:, ksl], wt[:], wh[:, ksl])
                return wh, wl

            # qT/kT = W^T @ xT  (out [1152(9 mt), 512]), emit fp16 hi/lo
            for w_ap, hiT, loT in ((wq, qhT, qlT), (wk, khT, klT)):
                wh, wl = w_pairs(w_ap)
                for mt in range(KT):
                    pt = ps.tile([P, CHUNK], F32, tag="pt")
                    msl = slice(mt * P, (mt + 1) * P)
                    for kk in range(KT):
                        nc.tensor.matmul(pt[:], wh[:, kk, msl], xh[:, kk],
                                         start=(kk == 0), stop=False)
                        nc.tensor.matmul(pt[:], wh[:, kk, msl], xl[:, kk],
                                         start=False, stop=False)
                        nc.tensor.matmul(pt[:], wl[:, kk, msl], xh[:, kk],
                                         start=False, stop=(kk == KT - 1))
                    hi = op.tile([P, CHUNK], F16, tag="hi")
                    lo = op.tile([P, CHUNK], F16, tag="lo")
                    nc.scalar.copy(hi[:], pt[:])
                    nc.vector.tensor_sub(lo[:], pt[:], hi[:])
                    nc.sync.dma_start(hiT[mt * P:(mt + 1) * P, :], hi[:])
                    nc.sync.dma_start(loT[mt * P:(mt + 1) * P, :], lo[:])

            # v = x_chunk @ Wv  (out [512(4 mt), 1152(3 x 384)])
            NS = 384
            whv, wlv = w_pairs(wv)
            vo = vp.tile([P, CHUNK // P, INNER], F32)
            for mt in range(CHUNK // P):
                xsl = slice(mt * P, (mt + 1) * P)
                for ns in range(INNER // NS):
                    pv = ps.tile([P, NS], F32, tag="pv")
                    nsl = slice(ns * NS, (ns + 1) * NS)
                    for kk in range(KT):
                        nc.tensor.matmul(pv[:], xh[:, kk, xsl], whv[:, kk, nsl],
                                         start=(kk == 0), stop=False)
                        nc.tensor.matmul(pv[:], xh[:, kk, xsl], wlv[:, kk, nsl],
                                         start=False, stop=False)
                        nc.tensor.matmul(pv[:], xl[:, kk, xsl], whv[:, kk, nsl],
                                         start=False, stop=(kk == KT - 1))
                    nc.scalar.copy(vo[:, mt, nsl], pv[:])
            nc.sync.dma_start(vout.rearrange("(mt p) m -> p mt m", p=P), vo[:])
    return split_waits(nc)


def run_proj(x, Wq, Wk, Wv, trace=False):
    """x [B,N,QD] f32 -> qhT,qlT,khT,klT [B][INNER,N] fp16 ; v [B][N,INNER] f32"""
    xf = np.ascontiguousarray(x.reshape(B * N, QD).T)  # [QD, 4096]
    in_maps = []
    for c in range(NC):
        in_maps.append({
            "xT": np.ascontiguousarray(xf[:, c * CHUNK:(c + 1) * CHUNK]),
            "wq": Wq, "wk": Wk, "wv": Wv,
        })
    res = run_bass_kernel_spmd(build_proj(), in_maps, core_ids=_CORE_IDS, trace=trace)
    outs = {}
    for name in ("qhT", "qlT", "khT", "klT"):
        full = np.concatenate([res.results[c][name] for c in range(NC)], axis=1)
        outs[name] = [full[:, b * N:(b + 1) * N] for b in range(B)]
    vfull = np.concatenate([res.results[c]["v"] for c in range(NC)], axis=0)
    outs["v"] = [vfull[b * N:(b + 1) * N] for b in range(B)]
    return outs, res


if __name__ == "__main__":
    import sys
    phase = sys.argv[1] if len(sys.argv) > 1 else "proj"
    rng = np.random.default_rng(0)
    if phase == "sel":
        import jax
        with jax.default_device(jax.devices("cpu")[0]):
            import reference as R
            inputs = {k: np.asarray(v) for k, v in R.setup_inputs().items()}
        x, Wq, Wk = inputs["x"], inputs["Wq"], inputs["Wk"]
        proj, _ = run_proj(x, Wq, Wk, inputs["Wv"])
        attn, _ = run_map(proj)
        sel, _ = run_sel(attn)
        q = (x.reshape(B * N, QD).astype(np.float64) @ Wq).reshape(B, N, H, D)
        k = (x.reshape(B * N, QD).astype(np.float64) @ Wk).reshape(B, N, H, D)
        for b in range(B):
            S = np.einsum('nhd,mhd->hnm', q[b], k[b]) * float(SCALE)
            E = np.exp(S)
            M = (E / E.sum(-1, keepdims=True)).mean(0)
            dist = np.full((1, N), 1.0 / N)
            for _ in range(5):
                dist = dist @ M
            imp = dist[0]
            order = np.argsort(-imp, kind='stable')
            keep_ref = np.zeros(N); keep_ref[order[:N_KEEP]] = 1
            got_keep = sel[b]["keep"]
            print(f"b={b} keep count={int(got_keep.sum())} "
                  f"mismatches={int((got_keep != keep_ref).sum())} "
                  f"imp err={np.abs(sel[b]['imp'] - imp).max():.2e}")
            kr = np.sort(order[:N_KEEP])
            src_ref = kr[M[kr].argmax(axis=0)]
            print(f"   src mismatches={int((sel[b]['src'] != src_ref).sum())}")
    if phase == "map":
        import jax
        with jax.default_device(jax.devices("cpu")[0]):
            import reference as R
            inputs = {k: np.asarray(v) for k, v in R.setup_inputs().items()}
        x, Wq, Wk = inputs["x"], inputs["Wq"], inputs["Wk"]
        proj, _ = run_proj(x, Wq, Wk, inputs["Wv"])
        attn, res = run_map(proj)
        # host check: f32 map from f64 ground truth
        q = (x.reshape(B * N, QD).astype(np.float64) @ Wq).reshape(B, N, H, D)
        k = (x.reshape(B * N, QD).astype(np.float64) @ Wk).reshape(B, N, H, D)
        for b in range(B):
            S = np.einsum('nhd,mhd->hnm', q[b], k[b]) * float(SCALE)
            E = np.exp(S)
            M = (E / E.sum(-1, keepdims=True)).mean(0)
            print(f"b={b} attn absmax err vs f64: {np.abs(attn[b] - M).max():.3e} "
                  f"(val scale {M.max():.3e})")
            dist = np.full((1, N), 1.0 / N)
            for _ in range(5):
                dist = dist @ M
            impd = dist[0]
            distf = np.full((1, N), np.float32(1.0 / N), np.float32)
            for _ in range(5):
                distf = (distf @ attn[b]).astype(np.float32)
            impf = distf[0]
            t64 = set(np.argsort(-impd)[:N_KEEP]); t32 = set(np.argsort(-impf)[:N_KEEP])
            keep = np.sort(np.argsort(-impd)[:N_KEEP])
            amd = M[keep].argmax(axis=0); amf = attn[b][keep].argmax(axis=0)
            print(f"   topk flips: {len(t64 ^ t32)}  argmax flips: {(amd != amf).sum()}")
    if phase == "proj":
        x = (rng.standard_normal((B, N, QD)) * 1.0).astype(np.float32)
        Wq = (rng.standard_normal((QD, INNER)) * 0.02).astype(np.float32)
        Wk = (rng.standard_normal((QD, INNER)) * 0.02).astype(np.float32)
        Wv = (rng.standard_normal((QD, INNER)) * 0.02).astype(np.float32)
        outs, res = run_proj(x, Wq, Wk, Wv)
        q = (x.reshape(B * N, QD) @ Wq).reshape(B, N, INNER)
        k = (x.reshape(B * N, QD) @ Wk).reshape(B, N, INNER)
        v = (x.reshape(B * N, QD) @ Wv).reshape(B, N, INNER)
        for b in range(B):
            qT = outs["qhT"][b].astype(np.float32) + outs["qlT"][b].astype(np.float32)
            kT = outs["khT"][b].astype(np.float32) + outs["klT"][b].astype(np.float32)
            print(f"b={b} q err {np.abs(qT.T - q[b]).max():.3e}"
                  f" k err {np.abs(kT.T - k[b]).max():.3e}"
                  f" v err {np.abs(outs['v'][b] - v[b]).max():.3e}"
                  f" (scale {np.abs(q[b]).max():.3f})")



# revision 25
# speedup vs baseline: 1.9196x; 1.9196x over previous
"""Trainium2 Bass kernel for nn_MemoryEfficientCrossAttention (WPR-pruned attention).

Self-contained: hardcodes shapes/sharding. The harness calls kernel(**inputs).

Pipeline (3 SPMD launches on 8 NeuronCores + host selection):
  P1 proj:  core c: tokens [c*512,(c+1)*512) of flattened x -> qT,kT fp16 hi/lo
            pairs ([1152,512] each) via 3-product fp16 pair matmuls, and
            vT fp16 ([1152,512]) via a single hi*hi product.
  P2 map:   core (b,j): query rows j*512..+512 of batch b, all 16 heads ->
            attn row-chunk [512,2048] f32 (mean over heads of softmax).
            The 3 pair products are packed into 2 stacked matmuls per
            128/88-row contraction tiles (exact same product set). Exp with
            -ln(16) constant bias folds the head-mean scale; weighted
            normalize-accumulate over heads split across DVE and GpSimd.
  Host sel: power iteration (f64) -> top-819 keep set; per-column argmax over
            retained rows -> recovery sources. Host gathers retained tokens.
  P3 attnr: core (b,j): retained-token quarter -> attention over retained
            keys, all fp16 single-pass (S^T, exp, PV with fused ones-column
            rowsums, /rowsum, @Wo + bo) -> finalT chunk, f32.
  Host: scatter retained rows, recovery copy final[prune] = final[src[prune]].
"""

import numpy as np

import concourse.bass as bass
import concourse.mybir as mybir
import concourse.tile as tile
from concourse.bass_utils import run_bass_kernel_spmd

F32 = mybir.dt.float32
F16 = mybir.dt.float16
AF = mybir.ActivationFunctionType

B, N, QD, H, D = 2, 2048, 1152, 16, 72
INNER = H * D
N_KEEP = 819
SCALE = np.float32(D ** -0.5)
P = 128
NC = 8
CHUNK = 512          # tokens per core in P1/P2
KT = QD // P         # 9 k-tiles of the 1152 contraction
NQP = 208            # padded retained tokens per quarter-core in P3
MP = 896             # padded retained-key count (819 -> 7 tiles of 128)
H_DVE = 9            # heads accumulated on DVE in P2 (rest on GpSimd)

_CORE_IDS = list(range(NC))


def split_waits(nc, maxw=1):
    """This toolchain's walrus accepts only one sync-wait per instruction;
    move excess waits onto preceding same-engine EventSemaphore nops."""
    n_new = 0
    for f in nc.m.functions:
        for blk in f.blocks:
            out = []
            changed = False
            for inst in blk.instructions:
                si = inst.sync_info
                if si is not None and si.on_wait is not None and len(si.on_wait) > maxw:
                    waits = list(si.on_wait)
                    for w in waits[:-maxw]:
                        es = mybir.InstEventSemaphore(
                            name=f"Wsplit{n_new}", ins=[], outs=[])
                        es.engine = inst.engine
                        es.sync_info = mybir.SyncInfo(on_wait=[w], on_update=[])
                        out.append(es)
                        n_new += 1
                    si.on_wait = waits[-maxw:]
                    changed = True
                out.append(inst)
            if changed:
                blk.instructions = out
    return nc


# --------------------------------------------------------------------------
# P1: projections.  per core: xT_chunk [1152, 512] -> qT/kT fp16 pairs, vT fp16.
# --------------------------------------------------------------------------

def build_proj():
    nc = bass.Bass("TRN2", target_bir_lowering=False, debug=False, num_devices=NC)
    xT = nc.dram_tensor("xT", [QD, CHUNK], F32, kind="ExternalInput").ap()
    wq = nc.dram_tensor("wq", [QD, INNER], F32, kind="ExternalInput").ap()
    wk = nc.dram_tensor("wk", [QD, INNER], F32, kind="ExternalInput").ap()
    wv = nc.dram_tensor("wv", [QD, INNER], F32, kind="ExternalInput").ap()
    qhT = nc.dram_tensor("qhT", [INNER, CHUNK], F16, kind="ExternalOutput").ap()
    qlT = nc.dram_tensor("qlT", [INNER, CHUNK], F16, kind="ExternalOutput").ap()
    khT = nc.dram_tensor("khT", [INNER, CHUNK], F16, kind="ExternalOutput").ap()
    klT = nc.dram_tensor("klT", [INNER, CHUNK], F16, kind="ExternalOutput").ap()
    vT = nc.dram_tensor("vT", [INNER, CHUNK], F16, kind="ExternalOutput").ap()

    with tile.TileContext(nc) as tc:
        with tc.tile_pool(name="xp", bufs=1) as xp, \
             tc.tile_pool(name="sbp", bufs=2) as sbp, \
             tc.tile_pool(name="wp", bufs=2) as wp, \
             tc.tile_pool(name="op", bufs=3) as op, \
             tc.tile_pool(name="ps", bufs=4, space="PSUM") as ps:
            # x pairs resident; f32 staged through sub-band tiles
            xh = xp.tile([P, KT, CHUNK], F16)
            xl = xp.tile([P, KT, CHUNK], F16)
            for sub in range(3):
                xt = sbp.tile([P, 3, CHUNK], F32, tag="xt")
                ksl = slice(sub * 3, sub * 3 + 3)
                nc.sync.dma_start(
                    xt[:], xT.rearrange("(kc p) m -> p kc m", p=P)[:, ksl])
                nc.scalar.copy(xh[:, ksl], xt[:])
                nc.vector.tensor_sub(xl[:, ksl], xt[:], xh[:, ksl])

            def w_pairs(w_ap, lo=True):
                wh = wp.tile([P, KT, INNER], F16, tag="wh")
                wl = None
                if lo:
                    wl = wp.tile([P, KT, INNER], F16, tag="wl")
                for sub in range(3):
                    wt = sbp.tile([P, 3, INNER], F32, tag="wt")
                    ksl = slice(sub * 3, sub * 3 + 3)
                    nc.sync.dma_start(
                        wt[:], w_ap.rearrange("(kc p) m -> p kc m", p=P)[:, ksl])
                    nc.scalar.copy(wh[:, ksl], wt[:])
                    if lo:
                        nc.vector.tensor_sub(wl[:, ksl], wt[:], wh[:, ksl])
                return wh, wl

            # qT/kT = W^T @ xT  (out [1152(9 mt), 512]), emit fp16 hi/lo
            for w_ap, hiT, loT in ((wq, qhT, qlT), (wk, khT, klT)):
                wh, wl = w_pairs(w_ap)
                for mt in range(KT):
                    pt = ps.tile([P, CHUNK], F32, tag="pt")
                    msl = slice(mt * P, (mt + 1) * P)
                    for kk in range(KT):
                        nc.tensor.matmul(pt[:], wh[:, kk, msl], xh[:, kk],
                                         start=(kk == 0), stop=False)
                        nc.tensor.matmul(pt[:], wh[:, kk, msl], xl[:, kk],
                                         start=False, stop=False)
                        nc.tensor.matmul(pt[:], wl[:, kk, msl], xh[:, kk],
                                         start=False, stop=(kk == KT - 1))
                    hi = op.tile([P, CHUNK], F16, tag="hi")
                    lo = op.tile([P, CHUNK], F16, tag="lo")
                    nc.scalar.copy(hi[:], pt[:])
                    nc.vector.tensor_sub(lo[:], pt[:], hi[:])
                    nc.sync.dma_start(hiT[mt * P:(mt + 1) * P, :], hi[:])
                    nc.sync.dma_start(loT[mt * P:(mt + 1) * P, :], lo[:])

            # vT = Wv^T @ xT, single fp16 product (output precision only)
            wvh, _ = w_pairs(wv, lo=False)
            for mt in range(KT):
                pv = ps.tile([P, CHUNK], F32, tag="pt")
                msl = slice(mt * P, (mt + 1) * P)
                for kk in range(KT):
                    nc.tensor.matmul(pv[:], wvh[:, kk, msl], xh[:, kk],
                                     start=(kk == 0), stop=(kk == KT - 1))
                vv = op.tile([P, CHUNK], F16, tag="hi")
                nc.scalar.copy(vv[:], pv[:])
                nc.sync.dma_start(vT[mt * P:(mt + 1) * P, :], vv[:])
    return split_waits(nc)


def run_proj(x, Wq, Wk, Wv, trace=False):
    """x [B,N,QD] f32 -> qhT,qlT,khT,klT,vT [B][INNER,N] fp16"""
    xf = np.ascontiguousarray(x.reshape(B * N, QD).T)  # [QD, 4096]
    in_maps = []
    for c in range(NC):
        in_maps.append({
            "xT": np.ascontiguousarray(xf[:, c * CHUNK:(c + 1) * CHUNK]),
            "wq": Wq, "wk": Wk, "wv": Wv,
        })
    res = run_bass_kernel_spmd(build_proj(), in_maps, core_ids=_CORE_IDS, trace=trace)
    outs = {}
    for name in ("qhT", "qlT", "khT", "klT", "vT"):
        full = np.concatenate([res.results[c][name] for c in range(NC)], axis=1)
        outs[name] = [full[:, b * N:(b + 1) * N] for b in range(B)]
    return outs, res


# --------------------------------------------------------------------------
# P2: attention map.  per core (b, j): query rows [j*512,(j+1)*512) of batch b
# -> attn row-chunk [512, 2048] f32 = mean over heads of row-softmax.
# The 3 fp16-pair products (qh.kh + qh.kl + ql.kh) are packed into 2 stacked
# contraction tiles of 128 and 88 rows:
#   tile A: rows [qh(0:72); qh(0:56)] x [kh(0:72); kl(0:56)]
#   tile B: rows [qh(56:72); ql(0:72)] x [kl(56:72); kh(0:72)]
# --------------------------------------------------------------------------

PB = 216 - P  # 88 rows in the second stacked tile


def build_map():
    nc = bass.Bass("TRN2", target_bir_lowering=False, debug=False, num_devices=NC)
    qh = nc.dram_tensor("qh", [INNER, CHUNK], F16, kind="ExternalInput").ap()
    ql = nc.dram_tensor("ql", [INNER, CHUNK], F16, kind="ExternalInput").ap()
    kh = nc.dram_tensor("kh", [INNER, N], F16, kind="ExternalInput").ap()
    kl = nc.dram_tensor("kl", [INNER, N], F16, kind="ExternalInput").ap()
    attn = nc.dram_tensor("attn", [CHUNK, N], F32, kind="ExternalOutput").ap()

    NSEG = N // 512  # 4 column segments per row
    qh_r = qh.rearrange("(h d) m -> d h m", d=D)
    ql_r = ql.rearrange("(h d) m -> d h m", d=D)
    kh_r = kh.rearrange("(h d) m -> d h m", d=D)
    kl_r = kl.rearrange("(h d) m -> d h m", d=D)
    with tile.TileContext(nc) as tc:
        with tc.tile_pool(name="kp", bufs=1) as kp, \
             tc.tile_pool(name="qp", bufs=2) as qp, \
             tc.tile_pool(name="ep", bufs=3) as ep, \
             tc.tile_pool(name="apA", bufs=2) as apA, \
             tc.tile_pool(name="sp", bufs=4) as sp, \
             tc.tile_pool(name="ps", bufs=2, space="PSUM") as ps:
            # resident stacked k tiles, one pair per head so a head's matmuls
            # only gate on their own small loads; q tiles per 128-query tile,
            # interleaved into the load order so the DMA queue serves the
            # compute front in consumption order.
            ksA = [None] * H
            ksB = [None] * H
            qsA = [None] * 4
            qsB = [None] * 4
            def load_k(h):
                hs = slice(h, h + 1)
                kA = kp.tile([P, 1, N], F16, name=f"ksA{h}")
                kB = kp.tile([PB, 1, N], F16, name=f"ksB{h}")
                nc.sync.dma_start(kA[0:D], kh_r[:, hs])
                nc.sync.dma_start(kA[D:P], kl_r[0:P - D, hs])
                nc.sync.dma_start(kB[0:D - (P - D)], kl_r[P - D:D, hs])
                nc.sync.dma_start(kB[D - (P - D):PB], kh_r[:, hs])
                ksA[h] = kA
                ksB[h] = kB
            def load_q(nqt):
                msl = slice(nqt * P, (nqt + 1) * P)
                qA = qp.tile([P, H, P], F16, tag="qsA")
                qB = qp.tile([PB, H, P], F16, tag="qsB")
                nc.sync.dma_start(qA[0:D], qh_r[:, :, msl])
                nc.sync.dma_start(qA[D:P], qh_r[0:P - D, :, msl])
                nc.sync.dma_start(qB[0:D - (P - D)], qh_r[P - D:D, :, msl])
                nc.sync.dma_start(qB[D - (P - D):PB], ql_r[:, :, msl])
                qsA[nqt] = qA
                qsB[nqt] = qB
            load_q(0)
            for h in range(4):
                load_k(h)
            load_q(1)
            for h in range(4, H):
                load_k(h)
            for nqt in range(CHUNK // P):
                qsl = slice(nqt * P, (nqt + 1) * P)
                accA = apA.tile([P, N], F32, tag="accA")
                for h in range(H):
                    et = ep.tile([P, N], F32, tag="et")
                    rs = sp.tile([P, 1], F32, tag="rs")
                    w = sp.tile([P, 1], F32, tag="w")
                    pt4 = ps.tile([P, N], F32, tag="pt4")
                    for ms in range(NSEG):
                        seg = slice(ms * 512, (ms + 1) * 512)
                        nc.tensor.matmul(pt4[:, seg], qsA[nqt][:, h],
                                         ksA[h][:, 0, seg],
                                         start=True, stop=False)
                        nc.tensor.matmul(pt4[:, seg], qsB[nqt][:, h],
                                         ksB[h][:, 0, seg],
                                         start=False, stop=True)
                    # attn output is H * mean-map (sum of per-head
                    # softmaxes); host selection is scale-invariant.
                    nc.scalar.activation(et[:], pt4[:], AF.Exp,
                                         scale=float(SCALE), accum_out=rs[:])
                    nc.vector.reciprocal(w[:], rs[:])
                    if h == 0:
                        nc.vector.tensor_scalar(
                            accA[:], et[:], w[:], scalar2=None,
                            op0=mybir.AluOpType.mult)
                    else:
                        nc.vector.scalar_tensor_tensor(
                            accA[:], et[:], w[:], accA[:],
                            op0=mybir.AluOpType.mult,
                            op1=mybir.AluOpType.add)
                nc.sync.dma_start(attn[qsl, :], accA[:])
                if nqt + 2 < CHUNK // P:
                    load_q(nqt + 2)
    return split_waits(nc)


def run_map(proj, trace=False):
    in_maps = []
    for c in range(NC):
        b, j = divmod(c, 4)
        sl = slice(j * CHUNK, (j + 1) * CHUNK)
        in_maps.append({
            "qh": np.ascontiguousarray(proj["qhT"][b][:, sl]),
            "ql": np.ascontiguousarray(proj["qlT"][b][:, sl]),
            "kh": proj["khT"][b], "kl": proj["klT"][b],
        })
    res = run_bass_kernel_spmd(build_map(), in_maps, core_ids=_CORE_IDS, trace=trace)
    attn = [np.concatenate([res.results[b * 4 + j]["attn"] for j in range(4)], axis=0)
            for b in range(B)]
    return attn, res


# --------------------------------------------------------------------------
# Host selection: 5-step power iteration (f64), strict top-819 by importance,
# per-column argmax over retained rows for recovery sources.
# --------------------------------------------------------------------------

def host_select(attn):
    sel = []
    for b in range(B):
        A = attn[b]
        dist = np.full((1, N), 1.0 / N, np.float64)
        A64 = A.astype(np.float64)
        for _ in range(5):
            dist = dist @ A64
        imp = dist[0]
        order = np.argsort(-imp, kind="stable")
        keep = np.sort(order[:N_KEEP])
        prune = np.sort(order[N_KEEP:])
        src = keep[A[keep].argmax(axis=0)]  # [N] most-attending retained row
        sel.append({"keep": keep, "prune": prune, "src": src})
    return sel


# --------------------------------------------------------------------------
# P3: retained attention + output projection, all fp16 single-pass.
# per core (b, j): ~205 retained tokens (host-gathered q columns, padded to
# NQP) -> finT [1152, NQP] = (softmax(qk) @ v / rowsum) @ Wo + bo, transposed.
# Padded keys have k=0 (exp->1) but v and the ones-column are 0, so they
# contribute nothing to PV or the rowsums; no masking needed.
# --------------------------------------------------------------------------

def build_attnr():
    nc = bass.Bass("TRN2", target_bir_lowering=False, debug=False, num_devices=NC)
    qs = nc.dram_tensor("qs", [INNER, NQP], F16, kind="ExternalInput").ap()
    ks = nc.dram_tensor("ks", [INNER, MP], F16, kind="ExternalInput").ap()
    v97 = nc.dram_tensor("v97", [MP // P, H, P, 97], F16,
                         kind="ExternalInput").ap()  # v cols 0..71, ones col 96
    wo = nc.dram_tensor("wo", [INNER, INNER], F16, kind="ExternalInput").ap()
    boc = nc.dram_tensor("boc", [P, KT], F32, kind="ExternalInput").ap()
    finT = nc.dram_tensor("finT", [INNER, NQP], F32, kind="ExternalOutput").ap()

    NT = MP // P
    with tile.TileContext(nc) as tc:
        with tc.tile_pool(name="kp", bufs=1) as kp, \
             tc.tile_pool(name="ep", bufs=3) as ep, \
             tc.tile_pool(name="np_", bufs=3) as np_, \
             tc.tile_pool(name="op", bufs=1) as op, \
             tc.tile_pool(name="cp", bufs=1) as cp, \
             tc.tile_pool(name="fp", bufs=2) as fp, \
             tc.tile_pool(name="psS", bufs=3, space="PSUM") as psS, \
             tc.tile_pool(name="psO", bufs=2, space="PSUM") as psO, \
             tc.tile_pool(name="psB", bufs=1, space="PSUM") as psB, \
             tc.tile_pool(name="psF", bufs=2, space="PSUM") as psF:
            kst = kp.tile([D, H, MP], F16)
            nc.sync.dma_start(kst[:], ks.rearrange("(h d) m -> d h m", d=D))
            qst = kp.tile([D, H, NQP], F16)
            nc.sync.dma_start(qst[:], qs.rearrange("(h d) m -> d h m", d=D))
            vr = kp.tile([P, NT, H, 97], F16)
            nc.sync.dma_start(vr[:], v97.rearrange("mc h p c -> p mc h c"))
            wot = kp.tile([P, KT, INNER], F16)
            nc.sync.dma_start(wot[:], wo.rearrange("(kt p) m -> p kt m", p=P))
            bo_sb = cp.tile([P, KT], F32)
            nc.sync.dma_start(bo_sb[:], boc[:])
            ones72 = cp.tile([1, D], F16)
            nc.vector.memset(ones72[:], 1.0)

            ON = op.tile([P, KT, NQP], F16)  # normalized PV output, hd-stacked
            for h in range(H):
                # S^T logits in mc-pair PSUM tiles (2*208*4B fits one bank)
                et = ep.tile([P, NT, NQP], F16, tag="et")
                for mp0 in range(0, NT, 2):
                    w2 = min(2, NT - mp0)
                    pss = psS.tile([P, 2, NQP], F32, tag="pss")
                    for mc in range(mp0, mp0 + w2):
                        nc.tensor.matmul(pss[:, mc - mp0],
                                         kst[:, h, mc * P:(mc + 1) * P],
                                         qst[:, h], start=True, stop=True)
                    nc.scalar.activation(et[:, mp0:mp0 + w2], pss[:, 0:w2],
                                         AF.Exp, scale=float(SCALE))
                po = psO.tile([97, NQP], F32, tag="po")
                for mc in range(NT):
                    nc.tensor.matmul(po[:], vr[:, mc, h], et[:, mc],
                                     start=(mc == 0), stop=(mc == NT - 1))
                # normalize: rows 0..71 / row 96 (the ones-column sums)
                inv = np_.tile([1, NQP], F16, tag="inv")
                with nc.allow_low_precision(reason="fp16 1/rowsum is plenty "
                                            "for the 2e-2 output tolerance"):
                    nc.vector.reciprocal(inv[:], po[96:97, :])
                pb = psB.tile([D, NQP], F32, tag="pb")
                nc.tensor.matmul(pb[:], ones72[:], inv[:], start=True, stop=True)
                binv = np_.tile([D, NQP], F16, tag="binv")
                nc.scalar.copy(binv[:], pb[:])
                onv = np_.tile([D, NQP], F16, tag="onv")
                nc.vector.tensor_mul(onv[:], po[0:D, :], binv[:])
                g0 = h * D
                t0, p0 = divmod(g0, P)
                n0 = min(D, P - p0)
                nc.sync.dma_start(ON[p0:p0 + n0, t0, :], onv[0:n0, :])
                if n0 < D:
                    nc.sync.dma_start(ON[0:D - n0, t0 + 1, :], onv[n0:D, :])

            # final^T = Wo^T @ ON + bo
            for mt in range(KT):
                pf = psF.tile([P, NQP], F32, tag="pf")
                for kt in range(KT):
                    nc.tensor.matmul(pf[:], wot[:, kt, mt * P:(mt + 1) * P],
                                     ON[:, kt, :],
                                     start=(kt == 0), stop=(kt == KT - 1))
                foa = fp.tile([P, NQP], F32, tag="foa")
                nc.scalar.activation(foa[:], pf[:], AF.Identity,
                                     bias=bo_sb[:, mt:mt + 1])
                nc.sync.dma_start(finT[mt * P:(mt + 1) * P, :], foa[:])
    return split_waits(nc)


def run_attnr(proj, sel, Wo, bo, trace=False):
    in_maps = []
    meta = []
    boc = np.ascontiguousarray(bo.reshape(KT, P).T.astype(np.float32))
    wo16 = np.ascontiguousarray(Wo.astype(np.float16))
    for c in range(NC):
        b, j = divmod(c, 4)
        idx = sel[b]["keep"]
        bounds = np.linspace(0, len(idx), 5).astype(int)
        my = idx[bounds[j]:bounds[j + 1]]
        meta.append(my)
        qg = np.zeros((INNER, NQP), np.float16)
        qg[:, :len(my)] = proj["qhT"][b][:, my]
        kg = np.zeros((INNER, MP), np.float16)
        kg[:, :len(idx)] = proj["khT"][b][:, idx]
        vsel = np.zeros((MP, INNER), np.float16)
        vsel[:len(idx)] = proj["vT"][b][:, idx].T
        v97 = np.zeros((MP // P, H, P, 97), np.float16)
        v97[..., :D] = (vsel.reshape(MP // P, P, H, D)).transpose(0, 2, 1, 3)
        v97[..., 96] = 0.0
        ones = np.zeros(MP, np.float16)
        ones[:len(idx)] = 1.0
        v97[..., 96] = ones.reshape(MP // P, P)[:, None, :].repeat(H, axis=1)
        in_maps.append({
            "qs": qg, "ks": kg,
            "v97": np.ascontiguousarray(v97),
            "wo": wo16, "boc": boc,
        })
    res = run_bass_kernel_spmd(build_attnr(), in_maps, core_ids=_CORE_IDS, trace=trace)
    out = np.zeros((B, N, INNER), np.float32)
    for c in range(NC):
        b = c // 4
        my = meta[c]
        out[b][my] = res.results[c]["finT"][:, :len(my)].T
    # recovery: pruned tokens copy their most-attending retained token's row
    for b in range(B):
        prune = sel[b]["prune"]
        out[b][prune] = out[b][sel[b]["src"][prune]]
    return out, res


def kernel(x, Wq, Wk, Wv, Wo, bo):
    proj, _ = run_proj(np.asarray(x, np.float32), np.asarray(Wq, np.float32),
                       np.asarray(Wk, np.float32), np.asarray(Wv, np.float32))
    attn, _ = run_map(proj)
    sel = host_select(attn)
    out, _ = run_attnr(proj, sel, np.asarray(Wo, np.float32),
                       np.asarray(bo, np.float32))
    return out


# revision 31
# speedup vs baseline: 2.0141x; 1.0492x over previous
"""Trainium2 Bass kernel for nn_MemoryEfficientCrossAttention (WPR-pruned attention).

Self-contained: hardcodes shapes/sharding. The harness calls kernel(**inputs).

Pipeline (3 SPMD launches on 8 NeuronCores + host selection):
  P1 proj:  core c: tokens [c*512,(c+1)*512) of flattened x -> qT,kT fp16 hi/lo
            pairs ([1152,512] each) via 3-product fp16 pair matmuls, and
            vT fp16 ([1152,512]) via a single hi*hi product.
  P2 map:   core (b,j): query rows j*512..+512 of batch b, all 16 heads ->
            attn row-chunk [512,2048] f32 = H * (mean over heads of softmax);
            the positive scale H is irrelevant to host selection. The 3 pair
            products are packed into 2 stacked matmuls per 128/88-row
            contraction tile pair (exact same product set, 2/3 the PE time).
            Exp+rowsum on ACT; weighted normalize-accumulate on DVE.
  Host sel: power iteration (f64) -> top-819 keep set; per-column argmax over
            retained rows -> recovery sources. Host gathers retained tokens.
  P3 attnr: core (b,j): retained-token quarter -> attention over retained
            keys, all fp16 single-pass (S^T, exp, PV with fused ones-column
            rowsums, /rowsum, @Wo + bo) -> finalT chunk, f32.
  Host: scatter retained rows, recovery copy final[prune] = final[src[prune]].
"""

import numpy as np

import concourse.bass as bass
import concourse.mybir as mybir
import concourse.tile as tile
from concourse.bass_utils import run_bass_kernel_spmd

F32 = mybir.dt.float32
F16 = mybir.dt.float16
AF = mybir.ActivationFunctionType

B, N, QD, H, D = 2, 2048, 1152, 16, 72
INNER = H * D
N_KEEP = 819
SCALE = np.float32(D ** -0.5)
P = 128
NC = 8
CHUNK = 512          # tokens per core in P1/P2
KT = QD // P         # 9 k-tiles of the 1152 contraction
NQP = 208            # padded retained tokens per quarter-core in P3
MP = 896             # padded retained-key count (819 -> 7 tiles of 128)

_CORE_IDS = list(range(NC))


def split_waits(nc, maxw=1):
    """This toolchain's walrus accepts only one sync-wait per instruction;
    move excess waits onto preceding same-engine EventSemaphore nops."""
    n_new = 0
    for f in nc.m.functions:
        for blk in f.blocks:
            out = []
            changed = False
            for inst in blk.instructions:
                si = inst.sync_info
                if si is not None and si.on_wait is not None and len(si.on_wait) > maxw:
                    waits = list(si.on_wait)
                    for w in waits[:-maxw]:
                        es = mybir.InstEventSemaphore(
                            name=f"Wsplit{n_new}", ins=[], outs=[])
                        es.engine = inst.engine
                        es.sync_info = mybir.SyncInfo(on_wait=[w], on_update=[])
                        out.append(es)
                        n_new += 1
                    si.on_wait = waits[-maxw:]
                    changed = True
                out.append(inst)
            if changed:
                blk.instructions = out
    return nc


# --------------------------------------------------------------------------
# P1: projections.  per core: xT_chunk [1152, 512] -> qT/kT fp16 pairs, vT fp16.
# --------------------------------------------------------------------------

def build_proj():
    nc = bass.Bass("TRN2", target_bir_lowering=False, debug=False, num_devices=NC)
    xT = nc.dram_tensor("xT", [QD, CHUNK], F32, kind="ExternalInput").ap()
    wq = nc.dram_tensor("wq", [QD, INNER], F32, kind="ExternalInput").ap()
    wk = nc.dram_tensor("wk", [QD, INNER], F32, kind="ExternalInput").ap()
    wv = nc.dram_tensor("wv", [QD, INNER], F32, kind="ExternalInput").ap()
    qhT = nc.dram_tensor("qhT", [INNER, CHUNK], F16, kind="ExternalOutput").ap()
    qlT = nc.dram_tensor("qlT", [INNER, CHUNK], F16, kind="ExternalOutput").ap()
    khT = nc.dram_tensor("khT", [INNER, CHUNK], F16, kind="ExternalOutput").ap()
    klT = nc.dram_tensor("klT", [INNER, CHUNK], F16, kind="ExternalOutput").ap()
    vT = nc.dram_tensor("vT", [INNER, CHUNK], F16, kind="ExternalOutput").ap()

    with tile.TileContext(nc) as tc:
        with tc.tile_pool(name="xp", bufs=1) as xp, \
             tc.tile_pool(name="sbp", bufs=2) as sbp, \
             tc.tile_pool(name="wp", bufs=2) as wp, \
             tc.tile_pool(name="op", bufs=3) as op, \
             tc.tile_pool(name="ps", bufs=4, space="PSUM") as ps:
            # x pairs resident; f32 staged through sub-band tiles.
            # x and wq sub-loads are interleaved so the first matmul only
            # gates on the first sub-band of each.
            xh = xp.tile([P, KT, CHUNK], F16)
            xl = xp.tile([P, KT, CHUNK], F16)

            def w_pairs(w_ap, lo=True, x_interleave=False):
                wh = wp.tile([P, KT, INNER], F16, tag="wh")
                wl = None
                if lo:
                    wl = wp.tile([P, KT, INNER], F16, tag="wl")
                for sub in range(3):
                    ksl = slice(sub * 3, sub * 3 + 3)
                    if x_interleave:
                        xt = sbp.tile([P, 3, CHUNK], F32, tag="xt")
                        nc.sync.dma_start(
                            xt[:],
                            xT.rearrange("(kc p) m -> p kc m", p=P)[:, ksl])
                        nc.scalar.copy(xh[:, ksl], xt[:])
                        nc.vector.tensor_sub(xl[:, ksl], xt[:], xh[:, ksl])
                    wt = sbp.tile([P, 3, INNER], F32, tag="wt")
                    nc.sync.dma_start(
                        wt[:], w_ap.rearrange("(kc p) m -> p kc m", p=P)[:, ksl])
                    nc.scalar.copy(wh[:, ksl], wt[:])
                    if lo:
                        nc.vector.tensor_sub(wl[:, ksl], wt[:], wh[:, ksl])
                return wh, wl

            # qT/kT = W^T @ xT  (out [1152(9 mt), 512]), emit fp16 hi/lo
            for w_ap, hiT, loT in ((wq, qhT, qlT), (wk, khT, klT)):
                wh, wl = w_pairs(w_ap, x_interleave=(w_ap is wq))
                for mt in range(KT):
                    pt = ps.tile([P, CHUNK], F32, tag="pt")
                    msl = slice(mt * P, (mt + 1) * P)
                    for kk in range(KT):
                        nc.tensor.matmul(pt[:], wh[:, kk, msl], xh[:, kk],
                                         start=(kk == 0), stop=False)
                        nc.tensor.matmul(pt[:], wh[:, kk, msl], xl[:, kk],
                                         start=False, stop=False)
                        nc.tensor.matmul(pt[:], wl[:, kk, msl], xh[:, kk],
                                         start=False, stop=(kk == KT - 1))
                    hi = op.tile([P, CHUNK], F16, tag="hi")
                    lo = op.tile([P, CHUNK], F16, tag="lo")
                    nc.scalar.copy(hi[:], pt[:])
                    nc.vector.tensor_sub(lo[:], pt[:], hi[:])
                    nc.sync.dma_start(hiT[mt * P:(mt + 1) * P, :], hi[:])
                    nc.sync.dma_start(loT[mt * P:(mt + 1) * P, :], lo[:])

            # vT = Wv^T @ xT, single fp16 product (output precision only)
            wvh, _ = w_pairs(wv, lo=False)
            for mt in range(KT):
                pv = ps.tile([P, CHUNK], F32, tag="pt")
                msl = slice(mt * P, (mt + 1) * P)
                for kk in range(KT):
                    nc.tensor.matmul(pv[:], wvh[:, kk, msl], xh[:, kk],
                                     start=(kk == 0), stop=(kk == KT - 1))
                vv = op.tile([P, CHUNK], F16, tag="hi")
                nc.scalar.copy(vv[:], pv[:])
                nc.sync.dma_start(vT[mt * P:(mt + 1) * P, :], vv[:])
    return split_waits(nc)


def run_proj(x, Wq, Wk, Wv, trace=False):
    """x [B,N,QD] f32 -> qhT,qlT,khT,klT,vT [B][INNER,N] fp16"""
    xf = np.ascontiguousarray(x.reshape(B * N, QD).T)  # [QD, 4096]
    in_maps = []
    for c in range(NC):
        in_maps.append({
            "xT": np.ascontiguousarray(xf[:, c * CHUNK:(c + 1) * CHUNK]),
            "wq": Wq, "wk": Wk, "wv": Wv,
        })
    res = run_bass_kernel_spmd(build_proj(), in_maps, core_ids=_CORE_IDS, trace=trace)
    outs = {}
    for name in ("qhT", "qlT", "khT", "klT", "vT"):
        full = np.concatenate([res.results[c][name] for c in range(NC)], axis=1)
        outs[name] = [full[:, b * N:(b + 1) * N] for b in range(B)]
    return outs, res


# --------------------------------------------------------------------------
# P2: attention map.  per core (b, j): query rows [j*512,(j+1)*512) of batch b
# -> attn row-chunk [512, 2048] f32 = mean over heads of row-softmax.
# The 3 fp16-pair products (qh.kh + qh.kl + ql.kh) are packed into 2 stacked
# contraction tiles of 128 and 88 rows:
#   tile A: rows [qh(0:72); qh(0:56)] x [kh(0:72); kl(0:56)]
#   tile B: rows [qh(56:72); ql(0:72)] x [kl(56:72); kh(0:72)]
# --------------------------------------------------------------------------

PB = 216 - P  # 88 rows in the second stacked tile


def build_map():
    nc = bass.Bass("TRN2", target_bir_lowering=False, debug=False, num_devices=NC)
    qh = nc.dram_tensor("qh", [INNER, CHUNK], F16, kind="ExternalInput").ap()
    ql = nc.dram_tensor("ql", [INNER, CHUNK], F16, kind="ExternalInput").ap()
    kh = nc.dram_tensor("kh", [INNER, N], F16, kind="ExternalInput").ap()
    kl = nc.dram_tensor("kl", [INNER, N], F16, kind="ExternalInput").ap()
    attn = nc.dram_tensor("attn", [CHUNK, N], F32, kind="ExternalOutput").ap()

    NSEG = N // 512  # 4 column segments per row
    qh_r = qh.rearrange("(h d) m -> d h m", d=D)
    ql_r = ql.rearrange("(h d) m -> d h m", d=D)
    kh_r = kh.rearrange("(h d) m -> d h m", d=D)
    kl_r = kl.rearrange("(h d) m -> d h m", d=D)
    with tile.TileContext(nc) as tc:
        with tc.tile_pool(name="kp", bufs=1) as kp, \
             tc.tile_pool(name="qp", bufs=2) as qp, \
             tc.tile_pool(name="ep", bufs=3) as ep, \
             tc.tile_pool(name="apA", bufs=2) as apA, \
             tc.tile_pool(name="sp", bufs=4) as sp, \
             tc.tile_pool(name="ps", bufs=2, space="PSUM") as ps:
            # resident stacked k tiles, one pair per head so a head's matmuls
            # only gate on their own small loads; q tiles per 128-query tile,
            # interleaved into the load order so the DMA queue serves the
            # compute front in consumption order.
            ksA = [None] * H
            ksB = [None] * H
            qsA = [None] * 4
            qsB = [None] * 4
            def load_k(h):
                hs = slice(h, h + 1)
                kA = kp.tile([P, 1, N], F16, name=f"ksA{h}")
                kB = kp.tile([PB, 1, N], F16, name=f"ksB{h}")
                nc.sync.dma_start(kA[0:D], kh_r[:, hs])
                nc.sync.dma_start(kA[D:P], kl_r[0:P - D, hs])
                nc.sync.dma_start(kB[0:D - (P - D)], kl_r[P - D:D, hs])
                nc.sync.dma_start(kB[D - (P - D):PB], kh_r[:, hs])
                ksA[h] = kA
                ksB[h] = kB
            def load_q(nqt):
                msl = slice(nqt * P, (nqt + 1) * P)
                qA = qp.tile([P, H, P], F16, tag="qsA")
                qB = qp.tile([PB, H, P], F16, tag="qsB")
                nc.sync.dma_start(qA[0:D], qh_r[:, :, msl])
                nc.sync.dma_start(qA[D:P], qh_r[0:P - D, :, msl])
                nc.sync.dma_start(qB[0:D - (P - D)], qh_r[P - D:D, :, msl])
                nc.sync.dma_start(qB[D - (P - D):PB], ql_r[:, :, msl])
                qsA[nqt] = qA
                qsB[nqt] = qB
            load_q(0)
            for h in range(4):
                load_k(h)
            load_q(1)
            for h in range(4, H):
                load_k(h)
            for nqt in range(CHUNK // P):
                qsl = slice(nqt * P, (nqt + 1) * P)
                accA = apA.tile([P, N], F32, tag="accA")
                for h in range(H):
                    et = ep.tile([P, N], F32, tag="et")
                    rs = sp.tile([P, 1], F32, tag="rs")
                    w = sp.tile([P, 1], F32, tag="w")
                    pt4 = ps.tile([P, N], F32, tag="pt4")
                    for ms in range(NSEG):
                        seg = slice(ms * 512, (ms + 1) * 512)
                        nc.tensor.matmul(pt4[:, seg], qsA[nqt][:, h],
                                         ksA[h][:, 0, seg],
                                         start=True, stop=False)
                        nc.tensor.matmul(pt4[:, seg], qsB[nqt][:, h],
                                         ksB[h][:, 0, seg],
                                         start=False, stop=True)
                    # attn output is H * mean-map (sum of per-head
                    # softmaxes); host selection is scale-invariant.
                    nc.scalar.activation(et[:], pt4[:], AF.Exp,
                                         scale=float(SCALE), accum_out=rs[:])
                    nc.vector.reciprocal(w[:], rs[:])
                    if h == 0:
                        nc.vector.tensor_scalar(
                            accA[:], et[:], w[:], scalar2=None,
                            op0=mybir.AluOpType.mult)
                    else:
                        nc.vector.scalar_tensor_tensor(
                            accA[:], et[:], w[:], accA[:],
                            op0=mybir.AluOpType.mult,
                            op1=mybir.AluOpType.add)
                nc.sync.dma_start(attn[qsl, :], accA[:])
                if nqt + 2 < CHUNK // P:
                    load_q(nqt + 2)
    return split_waits(nc)


def run_map(proj, trace=False):
    in_maps = []
    for c in range(NC):
        b, j = divmod(c, 4)
        sl = slice(j * CHUNK, (j + 1) * CHUNK)
        in_maps.append({
            "qh": np.ascontiguousarray(proj["qhT"][b][:, sl]),
            "ql": np.ascontiguousarray(proj["qlT"][b][:, sl]),
            "kh": proj["khT"][b], "kl": proj["klT"][b],
        })
    res = run_bass_kernel_spmd(build_map(), in_maps, core_ids=_CORE_IDS, trace=trace)
    attn = [np.concatenate([res.results[b * 4 + j]["attn"] for j in range(4)], axis=0)
            for b in range(B)]
    return attn, res


# --------------------------------------------------------------------------
# Host selection: 5-step power iteration (f64), strict top-819 by importance,
# per-column argmax over retained rows for recovery sources.
# --------------------------------------------------------------------------

def host_select(attn):
    sel = []
    for b in range(B):
        A = attn[b]
        dist = np.full((1, N), 1.0 / N, np.float64)
        A64 = A.astype(np.float64)
        for _ in range(5):
            dist = dist @ A64
        imp = dist[0]
        order = np.argsort(-imp, kind="stable")
        keep = np.sort(order[:N_KEEP])
        prune = np.sort(order[N_KEEP:])
        src = keep[A[keep].argmax(axis=0)]  # [N] most-attending retained row
        sel.append({"keep": keep, "prune": prune, "src": src})
    return sel


# --------------------------------------------------------------------------
# P3: retained attention + output projection, all fp16 single-pass.
# per core (b, j): ~205 retained tokens (host-gathered q columns, padded to
# NQP) -> finT [1152, NQP] = (softmax(qk) @ v / rowsum) @ Wo + bo, transposed.
# Padded keys have k=0 (exp->1) but v and the ones-column are 0, so they
# contribute nothing to PV or the rowsums; no masking needed.
# --------------------------------------------------------------------------

def build_attnr():
    nc = bass.Bass("TRN2", target_bir_lowering=False, debug=False, num_devices=NC)
    qs = nc.dram_tensor("qs", [INNER, NQP], F16, kind="ExternalInput").ap()
    ks = nc.dram_tensor("ks", [INNER, MP], F16, kind="ExternalInput").ap()
    v97 = nc.dram_tensor("v97", [P, MP // P, H, 97], F16,
                         kind="ExternalInput").ap()  # v cols 0..71, ones col 96
    wo = nc.dram_tensor("wo", [INNER, INNER], F16, kind="ExternalInput").ap()
    boc = nc.dram_tensor("boc", [P, KT], F32, kind="ExternalInput").ap()
    finT = nc.dram_tensor("finT", [INNER, NQP], F32, kind="ExternalOutput").ap()

    NT = MP // P
    with tile.TileContext(nc) as tc:
        with tc.tile_pool(name="kp", bufs=1) as kp, \
             tc.tile_pool(name="ep", bufs=3) as ep, \
             tc.tile_pool(name="np_", bufs=3) as np_, \
             tc.tile_pool(name="op", bufs=1) as op, \
             tc.tile_pool(name="cp", bufs=1) as cp, \
             tc.tile_pool(name="fp", bufs=2) as fp, \
             tc.tile_pool(name="psS", bufs=3, space="PSUM") as psS, \
             tc.tile_pool(name="psO", bufs=2, space="PSUM") as psO, \
             tc.tile_pool(name="psB", bufs=1, space="PSUM") as psB, \
             tc.tile_pool(name="psF", bufs=2, space="PSUM") as psF:
            ks_r = ks.rearrange("(h d) m -> d h m", d=D)
            qst = kp.tile([D, H, NQP], F16)
            nc.sync.dma_start(qst[:], qs.rearrange("(h d) m -> d h m", d=D))
            kst = []
            KG = 4
            for hg in range(H // KG):
                kt_h = kp.tile([D, KG, MP], F16, name=f"kst{hg}")
                nc.sync.dma_start(kt_h[:], ks_r[:, hg * KG:(hg + 1) * KG])
                kst.append(kt_h)
            vr = []
            for mc in range(NT):
                vt = kp.tile([P, H, 97], F16, name=f"vr{mc}")
                nc.sync.dma_start(vt[:], v97[:, mc])
                vr.append(vt)
            wot = kp.tile([P, KT, INNER], F16)
            nc.sync.dma_start(wot[:], wo.rearrange("(kt p) m -> p kt m", p=P))
            bo_sb = cp.tile([P, KT], F32)
            nc.sync.dma_start(bo_sb[:], boc[:])
            ones72 = cp.tile([1, D], F16)
            nc.vector.memset(ones72[:], 1.0)

            ON = op.tile([P, KT, NQP], F16)  # normalized PV output, hd-stacked
            for h in range(H):
                # S^T logits in mc-pair PSUM tiles (2*208*4B fits one bank)
                et = ep.tile([P, NT, NQP], F16, tag="et")
                for mp0 in range(0, NT, 2):
                    w2 = min(2, NT - mp0)
                    pss = psS.tile([P, 2, NQP], F32, tag="pss")
                    for mc in range(mp0, mp0 + w2):
                        nc.tensor.matmul(pss[:, mc - mp0],
                                         kst[h // 4][:, h % 4,
                                                     mc * P:(mc + 1) * P],
                                         qst[:, h], start=True, stop=True)
                    nc.scalar.activation(et[:, mp0:mp0 + w2], pss[:, 0:w2],
                                         AF.Exp, scale=float(SCALE))
                po = psO.tile([97, NQP], F32, tag="po")
                for mc in range(NT):
                    nc.tensor.matmul(po[:], vr[mc][:, h], et[:, mc],
                                     start=(mc == 0), stop=(mc == NT - 1))
                # normalize: rows 0..71 / row 96 (the ones-column sums)
                inv = np_.tile([1, NQP], F16, tag="inv")
                with nc.allow_low_precision(reason="fp16 1/rowsum is plenty "
                                            "for the 2e-2 output tolerance"):
                    nc.vector.reciprocal(inv[:], po[96:97, :])
                pb = psB.tile([D, NQP], F32, tag="pb")
                nc.tensor.matmul(pb[:], ones72[:], inv[:], start=True, stop=True)
                binv = np_.tile([D, NQP], F16, tag="binv")
                nc.vector.tensor_scalar(binv[:], pb[:], 1.0, scalar2=None,
                                        op0=mybir.AluOpType.mult)
                onv = np_.tile([D, NQP], F16, tag="onv")
                nc.vector.tensor_mul(onv[:], po[0:D, :], binv[:])
                g0 = h * D
                t0, p0 = divmod(g0, P)
                n0 = min(D, P - p0)
                nc.sync.dma_start(ON[p0:p0 + n0, t0, :], onv[0:n0, :])
                if n0 < D:
                    nc.sync.dma_start(ON[0:D - n0, t0 + 1, :], onv[n0:D, :])

            # final^T = Wo^T @ ON + bo
            for mt in range(KT):
                pf = psF.tile([P, NQP], F32, tag="pf")
                for kt in range(KT):
                    nc.tensor.matmul(pf[:], wot[:, kt, mt * P:(mt + 1) * P],
                                     ON[:, kt, :],
                                     start=(kt == 0), stop=(kt == KT - 1))
                foa = fp.tile([P, NQP], F32, tag="foa")
                nc.scalar.activation(foa[:], pf[:], AF.Identity,
                                     bias=bo_sb[:, mt:mt + 1])
                nc.sync.dma_start(finT[mt * P:(mt + 1) * P, :], foa[:])
    return split_waits(nc)


def run_attnr(proj, sel, Wo, bo, trace=False):
    in_maps = []
    meta = []
    boc = np.ascontiguousarray(bo.reshape(KT, P).T.astype(np.float32))
    wo16 = np.ascontiguousarray(Wo.astype(np.float16))
    for c in range(NC):
        b, j = divmod(c, 4)
        idx = sel[b]["keep"]
        bounds = np.linspace(0, len(idx), 5).astype(int)
        my = idx[bounds[j]:bounds[j + 1]]
        meta.append(my)
        qg = np.zeros((INNER, NQP), np.float16)
        qg[:, :len(my)] = proj["qhT"][b][:, my]
        kg = np.zeros((INNER, MP), np.float16)
        kg[:, :len(idx)] = proj["khT"][b][:, idx]
        vsel = np.zeros((MP, INNER), np.float16)
        vsel[:len(idx)] = proj["vT"][b][:, idx].T
        v97 = np.zeros((P, MP // P, H, 97), np.float16)
        v97[..., :D] = (vsel.reshape(MP // P, P, H, D)).transpose(1, 0, 2, 3)
        ones = np.zeros(MP, np.float16)
        ones[:len(idx)] = 1.0
        v97[..., 96] = ones.reshape(MP // P, P).T[:, :, None]
        in_maps.append({
            "qs": qg, "ks": kg,
            "v97": np.ascontiguousarray(v97),
            "wo": wo16, "boc": boc,
        })
    res = run_bass_kernel_spmd(build_attnr(), in_maps, core_ids=_CORE_IDS, trace=trace)
    out = np.zeros((B, N, INNER), np.float32)
    for c in range(NC):
        b = c // 4
        my = meta[c]
        out[b][my] = res.results[c]["finT"][:, :len(my)].T
    # recovery: pruned tokens copy their most-attending retained token's row
    for b in range(B):
        prune = sel[b]["prune"]
        out[b][prune] = out[b][sel[b]["src"][prune]]
    return out, res


def kernel(x, Wq, Wk, Wv, Wo, bo):
    proj, _ = run_proj(np.asarray(x, np.float32), np.asarray(Wq, np.float32),
                       np.asarray(Wk, np.float32), np.asarray(Wv, np.float32))
    attn, _ = run_map(proj)
    sel = host_select(attn)
    out, _ = run_attnr(proj, sel, np.asarray(Wo, np.float32),
                       np.asarray(bo, np.float32))
    return out


# revision 32
# speedup vs baseline: 2.0939x; 1.0396x over previous
"""Trainium2 Bass kernel for nn_MemoryEfficientCrossAttention (WPR-pruned attention).

Self-contained: hardcodes shapes/sharding. The harness calls kernel(**inputs).

Pipeline (3 SPMD launches on 8 NeuronCores + host selection):
  P1 proj:  core c: tokens [c*512,(c+1)*512) of flattened x -> qT,kT fp16 hi/lo
            pairs ([1152,512] each) via 3-product fp16 pair matmuls (x and W
            pairs are host-prepared fp16 casts), and vT fp16 via hi*hi.
  P2 map:   core (b,j): query rows j*512..+512 of batch b, all 16 heads ->
            attn row-chunk [512,2048] f32 = H * (mean over heads of softmax);
            the positive scale H is irrelevant to host selection. The 3 pair
            products are packed into 2 stacked matmuls per 128/88-row
            contraction tile pair (exact same product set, 2/3 the PE time).
            Exp+rowsum on ACT; weighted normalize-accumulate on DVE.
  Host sel: power iteration (f64) -> top-819 keep set; per-column argmax over
            retained rows -> recovery sources. Host gathers retained tokens.
  P3 attnr: core (b,j): retained-token quarter -> attention over retained
            keys, all fp16 single-pass (S^T, exp, PV with fused ones-column
            rowsums, /rowsum, @Wo + bo) -> finalT chunk, f32.
  Host: scatter retained rows, recovery copy final[prune] = final[src[prune]].
"""

import numpy as np

import concourse.bass as bass
import concourse.mybir as mybir
import concourse.tile as tile
from concourse.bass_utils import run_bass_kernel_spmd

F32 = mybir.dt.float32
F16 = mybir.dt.float16
AF = mybir.ActivationFunctionType

B, N, QD, H, D = 2, 2048, 1152, 16, 72
INNER = H * D
N_KEEP = 819
SCALE = np.float32(D ** -0.5)
P = 128
NC = 8
CHUNK = 512          # tokens per core in P1/P2
KT = QD // P         # 9 k-tiles of the 1152 contraction
NQP = 208            # padded retained tokens per quarter-core in P3
MP = 896             # padded retained-key count (819 -> 7 tiles of 128)

_CORE_IDS = list(range(NC))


def split_waits(nc, maxw=1):
    """This toolchain's walrus accepts only one sync-wait per instruction;
    move excess waits onto preceding same-engine EventSemaphore nops."""
    n_new = 0
    for f in nc.m.functions:
        for blk in f.blocks:
            out = []
            changed = False
            for inst in blk.instructions:
                si = inst.sync_info
                if si is not None and si.on_wait is not None and len(si.on_wait) > maxw:
                    waits = list(si.on_wait)
                    for w in waits[:-maxw]:
                        es = mybir.InstEventSemaphore(
                            name=f"Wsplit{n_new}", ins=[], outs=[])
                        es.engine = inst.engine
                        es.sync_info = mybir.SyncInfo(on_wait=[w], on_update=[])
                        out.append(es)
                        n_new += 1
                    si.on_wait = waits[-maxw:]
                    changed = True
                out.append(inst)
            if changed:
                blk.instructions = out
    return nc


# --------------------------------------------------------------------------
# P1: projections.  per core: xT_chunk [1152, 512] -> qT/kT fp16 pairs, vT fp16.
# --------------------------------------------------------------------------

def build_proj():
    nc = bass.Bass("TRN2", target_bir_lowering=False, debug=False, num_devices=NC)
    xT = nc.dram_tensor("xT", [QD, CHUNK], F32, kind="ExternalInput").ap()
    wq = nc.dram_tensor("wq", [QD, INNER], F32, kind="ExternalInput").ap()
    wk = nc.dram_tensor("wk", [QD, INNER], F32, kind="ExternalInput").ap()
    wv = nc.dram_tensor("wv", [QD, INNER], F32, kind="ExternalInput").ap()
    qhT = nc.dram_tensor("qhT", [INNER, CHUNK], F16, kind="ExternalOutput").ap()
    qlT = nc.dram_tensor("qlT", [INNER, CHUNK], F16, kind="ExternalOutput").ap()
    khT = nc.dram_tensor("khT", [INNER, CHUNK], F16, kind="ExternalOutput").ap()
    klT = nc.dram_tensor("klT", [INNER, CHUNK], F16, kind="ExternalOutput").ap()
    vT = nc.dram_tensor("vT", [INNER, CHUNK], F16, kind="ExternalOutput").ap()

    with tile.TileContext(nc) as tc:
        with tc.tile_pool(name="xp", bufs=1) as xp, \
             tc.tile_pool(name="sbp", bufs=2) as sbp, \
             tc.tile_pool(name="wp", bufs=2) as wp, \
             tc.tile_pool(name="op", bufs=3) as op, \
             tc.tile_pool(name="ps", bufs=4, space="PSUM") as ps:
            # x pairs resident; f32 staged through sub-band tiles.
            # x and wq sub-loads are interleaved so the first matmul only
            # gates on the first sub-band of each.
            xh = xp.tile([P, KT, CHUNK], F16)
            xl = xp.tile([P, KT, CHUNK], F16)

            def w_pairs(w_ap, lo=True, x_interleave=False):
                wh = wp.tile([P, KT, INNER], F16, tag="wh")
                wl = None
                if lo:
                    wl = wp.tile([P, KT, INNER], F16, tag="wl")
                for sub in range(3):
                    ksl = slice(sub * 3, sub * 3 + 3)
                    if x_interleave:
                        xt = sbp.tile([P, 3, CHUNK], F32, tag="xt")
                        nc.sync.dma_start(
                            xt[:],
                            xT.rearrange("(kc p) m -> p kc m", p=P)[:, ksl])
                        nc.scalar.copy(xh[:, ksl], xt[:])
                        nc.vector.tensor_sub(xl[:, ksl], xt[:], xh[:, ksl])
                    wt = sbp.tile([P, 3, INNER], F32, tag="wt")
                    nc.sync.dma_start(
                        wt[:], w_ap.rearrange("(kc p) m -> p kc m", p=P)[:, ksl])
                    nc.scalar.copy(wh[:, ksl], wt[:])
                    if lo:
                        nc.vector.tensor_sub(wl[:, ksl], wt[:], wh[:, ksl])
                return wh, wl

            # qT/kT = W^T @ xT  (out [1152(9 mt), 512]), emit fp16 hi/lo
            for w_ap, hiT, loT in ((wq, qhT, qlT), (wk, khT, klT)):
                wh, wl = w_pairs(w_ap, x_interleave=(w_ap is wq))
                for mt in range(KT):
                    pt = ps.tile([P, CHUNK], F32, tag="pt")
                    msl = slice(mt * P, (mt + 1) * P)
                    for kk in range(KT):
                        nc.tensor.matmul(pt[:], wh[:, kk, msl], xh[:, kk],
                                         start=(kk == 0), stop=False)
                        nc.tensor.matmul(pt[:], wh[:, kk, msl], xl[:, kk],
                                         start=False, stop=False)
                        nc.tensor.matmul(pt[:], wl[:, kk, msl], xh[:, kk],
                                         start=False, stop=(kk == KT - 1))
                    hi = op.tile([P, CHUNK], F16, tag="hi")
                    lo = op.tile([P, CHUNK], F16, tag="lo")
                    nc.scalar.copy(hi[:], pt[:])
                    nc.vector.tensor_sub(lo[:], pt[:], hi[:])
                    nc.sync.dma_start(hiT[mt * P:(mt + 1) * P, :], hi[:])
                    nc.sync.dma_start(loT[mt * P:(mt + 1) * P, :], lo[:])

            # vT = Wv^T @ xT, single fp16 product (output precision only)
            wvh, _ = w_pairs(wv, lo=False)
            for mt in range(KT):
                pv = ps.tile([P, CHUNK], F32, tag="pt")
                msl = slice(mt * P, (mt + 1) * P)
                for kk in range(KT):
                    nc.tensor.matmul(pv[:], wvh[:, kk, msl], xh[:, kk],
                                     start=(kk == 0), stop=(kk == KT - 1))
                vv = op.tile([P, CHUNK], F16, tag="hi")
                nc.scalar.copy(vv[:], pv[:])
                nc.sync.dma_start(vT[mt * P:(mt + 1) * P, :], vv[:])
    return split_waits(nc)


def run_proj(x, Wq, Wk, Wv, trace=False):
    """x [B,N,QD] f32 -> qhT,qlT,khT,klT,vT [B][INNER,N] fp16"""
    xf = np.ascontiguousarray(x.reshape(B * N, QD).T)  # [QD, 4096]
    in_maps = []
    for c in range(NC):
        in_maps.append({
            "xT": np.ascontiguousarray(xf[:, c * CHUNK:(c + 1) * CHUNK]),
            "wq": Wq, "wk": Wk, "wv": Wv,
        })
    res = run_bass_kernel_spmd(build_proj(), in_maps, core_ids=_CORE_IDS, trace=trace)
    outs = {}
    for name in ("qhT", "qlT", "khT", "klT", "vT"):
        full = np.concatenate([res.results[c][name] for c in range(NC)], axis=1)
        outs[name] = [full[:, b * N:(b + 1) * N] for b in range(B)]
    return outs, res


# --------------------------------------------------------------------------
# P2: attention map.  per core (b, j): query rows [j*512,(j+1)*512) of batch b
# -> attn row-chunk [512, 2048] f32 = mean over heads of row-softmax.
# The 3 fp16-pair products (qh.kh + qh.kl + ql.kh) are packed into 2 stacked
# contraction tiles of 128 and 88 rows:
#   tile A: rows [qh(0:72); qh(0:56)] x [kh(0:72); kl(0:56)]
#   tile B: rows [qh(56:72); ql(0:72)] x [kl(56:72); kh(0:72)]
# --------------------------------------------------------------------------

PB = 216 - P  # 88 rows in the second stacked tile


def build_map():
    nc = bass.Bass("TRN2", target_bir_lowering=False, debug=False, num_devices=NC)
    qh = nc.dram_tensor("qh", [INNER, CHUNK], F16, kind="ExternalInput").ap()
    ql = nc.dram_tensor("ql", [INNER, CHUNK], F16, kind="ExternalInput").ap()
    kh = nc.dram_tensor("kh", [INNER, N], F16, kind="ExternalInput").ap()
    kl = nc.dram_tensor("kl", [INNER, N], F16, kind="ExternalInput").ap()
    attn = nc.dram_tensor("attn", [CHUNK, N], F32, kind="ExternalOutput").ap()

    NSEG = N // 512  # 4 column segments per row
    qh_r = qh.rearrange("(h d) m -> d h m", d=D)
    ql_r = ql.rearrange("(h d) m -> d h m", d=D)
    kh_r = kh.rearrange("(h d) m -> d h m", d=D)
    kl_r = kl.rearrange("(h d) m -> d h m", d=D)
    with tile.TileContext(nc) as tc:
        with tc.tile_pool(name="kp", bufs=1) as kp, \
             tc.tile_pool(name="qp", bufs=2) as qp, \
             tc.tile_pool(name="ep", bufs=3) as ep, \
             tc.tile_pool(name="apA", bufs=2) as apA, \
             tc.tile_pool(name="sp", bufs=4) as sp, \
             tc.tile_pool(name="ps", bufs=2, space="PSUM") as ps:
            # resident stacked k tiles, one pair per head so a head's matmuls
            # only gate on their own small loads; q tiles per 128-query tile,
            # interleaved into the load order so the DMA queue serves the
            # compute front in consumption order.
            ksA = [None] * H
            ksB = [None] * H
            qsA = [None] * 4
            qsB = [None] * 4
            def load_k(h):
                hs = slice(h, h + 1)
                kA = kp.tile([P, 1, N], F16, name=f"ksA{h}")
                kB = kp.tile([PB, 1, N], F16, name=f"ksB{h}")
                nc.sync.dma_start(kA[0:D], kh_r[:, hs])
                nc.sync.dma_start(kA[D:P], kl_r[0:P - D, hs])
                nc.sync.dma_start(kB[0:D - (P - D)], kl_r[P - D:D, hs])
                nc.sync.dma_start(kB[D - (P - D):PB], kh_r[:, hs])
                ksA[h] = kA
                ksB[h] = kB
            def load_q(nqt):
                msl = slice(nqt * P, (nqt + 1) * P)
                qA = qp.tile([P, H, P], F16, tag="qsA")
                qB = qp.tile([PB, H, P], F16, tag="qsB")
                nc.sync.dma_start(qA[0:D], qh_r[:, :, msl])
                nc.sync.dma_start(qA[D:P], qh_r[0:P - D, :, msl])
                nc.sync.dma_start(qB[0:D - (P - D)], qh_r[P - D:D, :, msl])
                nc.sync.dma_start(qB[D - (P - D):PB], ql_r[:, :, msl])
                qsA[nqt] = qA
                qsB[nqt] = qB
            load_q(0)
            for h in range(4):
                load_k(h)
            load_q(1)
            for h in range(4, H):
                load_k(h)
            for nqt in range(CHUNK // P):
                qsl = slice(nqt * P, (nqt + 1) * P)
                accA = apA.tile([P, N], F32, tag="accA")
                for h in range(H):
                    et = ep.tile([P, N], F32, tag="et")
                    rs = sp.tile([P, 1], F32, tag="rs")
                    w = sp.tile([P, 1], F32, tag="w")
                    pt4 = ps.tile([P, N], F32, tag="pt4")
                    for ms in range(NSEG):
                        seg = slice(ms * 512, (ms + 1) * 512)
                        nc.tensor.matmul(pt4[:, seg], qsA[nqt][:, h],
                                         ksA[h][:, 0, seg],
                                         start=True, stop=False)
                        nc.tensor.matmul(pt4[:, seg], qsB[nqt][:, h],
                                         ksB[h][:, 0, seg],
                                         start=False, stop=True)
                    # attn output is H * mean-map (sum of per-head
                    # softmaxes); host selection is scale-invariant.
                    nc.scalar.activation(et[:], pt4[:], AF.Exp,
                                         scale=float(SCALE), accum_out=rs[:])
                    nc.vector.reciprocal(w[:], rs[:])
                    if h == 0:
                        nc.vector.tensor_scalar(
                            accA[:], et[:], w[:], scalar2=None,
                            op0=mybir.AluOpType.mult)
                    else:
                        nc.vector.scalar_tensor_tensor(
                            accA[:], et[:], w[:], accA[:],
                            op0=mybir.AluOpType.mult,
                            op1=mybir.AluOpType.add)
                nc.sync.dma_start(attn[qsl, :], accA[:])
                if nqt + 2 < CHUNK // P:
                    load_q(nqt + 2)
    return split_waits(nc)


def run_map(proj, trace=False):
    in_maps = []
    for c in range(NC):
        b, j = divmod(c, 4)
        sl = slice(j * CHUNK, (j + 1) * CHUNK)
        in_maps.append({
            "qh": np.ascontiguousarray(proj["qhT"][b][:, sl]),
            "ql": np.ascontiguousarray(proj["qlT"][b][:, sl]),
            "kh": proj["khT"][b], "kl": proj["klT"][b],
        })
    res = run_bass_kernel_spmd(build_map(), in_maps, core_ids=_CORE_IDS, trace=trace)
    attn = [np.concatenate([res.results[b * 4 + j]["attn"] for j in range(4)], axis=0)
            for b in range(B)]
    return attn, res


# --------------------------------------------------------------------------
# Host selection: 5-step power iteration (f64), strict top-819 by importance,
# per-column argmax over retained rows for recovery sources.
# --------------------------------------------------------------------------

def host_select(attn):
    sel = []
    for b in range(B):
        A = attn[b]
        dist = np.full((1, N), 1.0 / N, np.float64)
        A64 = A.astype(np.float64)
        for _ in range(5):
            dist = dist @ A64
        imp = dist[0]
        order = np.argsort(-imp, kind="stable")
        keep = np.sort(order[:N_KEEP])
        prune = np.sort(order[N_KEEP:])
        src = keep[A[keep].argmax(axis=0)]  # [N] most-attending retained row
        sel.append({"keep": keep, "prune": prune, "src": src})
    return sel


# --------------------------------------------------------------------------
# P3: retained attention + output projection, all fp16 single-pass.
# per core (b, j): ~205 retained tokens (host-gathered q columns, padded to
# NQP) -> finT [1152, NQP] = (softmax(qk) @ v / rowsum) @ Wo + bo, transposed.
# Padded keys have k=0 (exp->1) but v and the ones-column are 0, so they
# contribute nothing to PV or the rowsums; no masking needed.
# --------------------------------------------------------------------------

def build_attnr():
    nc = bass.Bass("TRN2", target_bir_lowering=False, debug=False, num_devices=NC)
    qs = nc.dram_tensor("qs", [INNER, NQP], F16, kind="ExternalInput").ap()
    ks = nc.dram_tensor("ks", [INNER, MP], F16, kind="ExternalInput").ap()
    v97 = nc.dram_tensor("v97", [P, MP // P, H, 97], F16,
                         kind="ExternalInput").ap()  # v cols 0..71, ones col 96
    wo = nc.dram_tensor("wo", [INNER, INNER], F16, kind="ExternalInput").ap()
    boc = nc.dram_tensor("boc", [P, KT], F32, kind="ExternalInput").ap()
    finT = nc.dram_tensor("finT", [INNER, NQP], F32, kind="ExternalOutput").ap()

    NT = MP // P
    with tile.TileContext(nc) as tc:
        with tc.tile_pool(name="kp", bufs=1) as kp, \
             tc.tile_pool(name="ep", bufs=3) as ep, \
             tc.tile_pool(name="np_", bufs=3) as np_, \
             tc.tile_pool(name="op", bufs=1) as op, \
             tc.tile_pool(name="cp", bufs=1) as cp, \
             tc.tile_pool(name="fp", bufs=2) as fp, \
             tc.tile_pool(name="psS", bufs=3, space="PSUM") as psS, \
             tc.tile_pool(name="psO", bufs=2, space="PSUM") as psO, \
             tc.tile_pool(name="psB", bufs=1, space="PSUM") as psB, \
             tc.tile_pool(name="psF", bufs=2, space="PSUM") as psF:
            ks_r = ks.rearrange("(h d) m -> d h m", d=D)
            qst = kp.tile([D, H, NQP], F16)
            nc.sync.dma_start(qst[:], qs.rearrange("(h d) m -> d h m", d=D))
            kst = []
            KG = 4
            for hg in range(H // KG):
                kt_h = kp.tile([D, KG, MP], F16, name=f"kst{hg}")
                nc.sync.dma_start(kt_h[:], ks_r[:, hg * KG:(hg + 1) * KG])
                kst.append(kt_h)
            vr = []
            for mc in range(NT):
                vt = kp.tile([P, H, 97], F16, name=f"vr{mc}")
                nc.sync.dma_start(vt[:], v97[:, mc])
                vr.append(vt)
            wot = kp.tile([P, KT, INNER], F16)
            nc.sync.dma_start(wot[:], wo.rearrange("(kt p) m -> p kt m", p=P))
            bo_sb = cp.tile([P, KT], F32)
            nc.sync.dma_start(bo_sb[:], boc[:])
            ones72 = cp.tile([1, D], F16)
            nc.vector.memset(ones72[:], 1.0)

            ON = op.tile([P, KT, NQP], F16)  # normalized PV output, hd-stacked
            for h in range(H):
                # S^T logits in mc-pair PSUM tiles (2*208*4B fits one bank)
                et = ep.tile([P, NT, NQP], F16, tag="et")
                for mp0 in range(0, NT, 2):
                    w2 = min(2, NT - mp0)
                    pss = psS.tile([P, 2, NQP], F32, tag="pss")
                    for mc in range(mp0, mp0 + w2):
                        nc.tensor.matmul(pss[:, mc - mp0],
                                         kst[h // 4][:, h % 4,
                                                     mc * P:(mc + 1) * P],
                                         qst[:, h], start=True, stop=True)
                    nc.scalar.activation(et[:, mp0:mp0 + w2], pss[:, 0:w2],
                                         AF.Exp, scale=float(SCALE))
                po = psO.tile([97, NQP], F32, tag="po")
                for mc in range(NT):
                    nc.tensor.matmul(po[:], vr[mc][:, h], et[:, mc],
                                     start=(mc == 0), stop=(mc == NT - 1))
                # normalize: rows 0..71 / row 96 (the ones-column sums)
                inv = np_.tile([1, NQP], F16, tag="inv")
                with nc.allow_low_precision(reason="fp16 1/rowsum is plenty "
                                            "for the 2e-2 output tolerance"):
                    nc.vector.reciprocal(inv[:], po[96:97, :])
                pb = psB.tile([D, NQP], F32, tag="pb")
                nc.tensor.matmul(pb[:], ones72[:], inv[:], start=True, stop=True)
                binv = np_.tile([D, NQP], F16, tag="binv")
                nc.vector.tensor_scalar(binv[:], pb[:], 1.0, scalar2=None,
                                        op0=mybir.AluOpType.mult)
                onv = np_.tile([D, NQP], F16, tag="onv")
                nc.vector.tensor_mul(onv[:], po[0:D, :], binv[:])
                g0 = h * D
                t0, p0 = divmod(g0, P)
                n0 = min(D, P - p0)
                nc.sync.dma_start(ON[p0:p0 + n0, t0, :], onv[0:n0, :])
                if n0 < D:
                    nc.sync.dma_start(ON[0:D - n0, t0 + 1, :], onv[n0:D, :])

            # final^T = Wo^T @ ON + bo
            for mt in range(KT):
                pf = psF.tile([P, NQP], F32, tag="pf")
                for kt in range(KT):
                    nc.tensor.matmul(pf[:], wot[:, kt, mt * P:(mt + 1) * P],
                                     ON[:, kt, :],
                                     start=(kt == 0), stop=(kt == KT - 1))
                foa = fp.tile([P, NQP], F32, tag="foa")
                nc.scalar.activation(foa[:], pf[:], AF.Identity,
                                     bias=bo_sb[:, mt:mt + 1])
                nc.sync.dma_start(finT[mt * P:(mt + 1) * P, :], foa[:])
    return split_waits(nc)


def run_attnr(proj, sel, Wo, bo, trace=False):
    in_maps = []
    meta = []
    boc = np.ascontiguousarray(bo.reshape(KT, P).T.astype(np.float32))
    wo16 = np.ascontiguousarray(Wo.astype(np.float16))
    for c in range(NC):
        b, j = divmod(c, 4)
        idx = sel[b]["keep"]
        bounds = np.linspace(0, len(idx), 5).astype(int)
        my = idx[bounds[j]:bounds[j + 1]]
        meta.append(my)
        qg = np.zeros((INNER, NQP), np.float16)
        qg[:, :len(my)] = proj["qhT"][b][:, my]
        kg = np.zeros((INNER, MP), np.float16)
        kg[:, :len(idx)] = proj["khT"][b][:, idx]
        vsel = np.zeros((MP, INNER), np.float16)
        vsel[:len(idx)] = proj["vT"][b][:, idx].T
        v97 = np.zeros((P, MP // P, H, 97), np.float16)
        v97[..., :D] = (vsel.reshape(MP // P, P, H, D)).transpose(1, 0, 2, 3)
        ones = np.zeros(MP, np.float16)
        ones[:len(idx)] = 1.0
        v97[..., 96] = ones.reshape(MP // P, P).T[:, :, None]
        in_maps.append({
            "qs": qg, "ks": kg,
            "v97": np.ascontiguousarray(v97),
            "wo": wo16, "boc": boc,
        })
    res = run_bass_kernel_spmd(build_attnr(), in_maps, core_ids=_CORE_IDS, trace=trace)
    out = np.zeros((B, N, INNER), np.float32)
    for c in range(NC):
        b = c // 4
        my = meta[c]
        out[b][my] = res.results[c]["finT"][:, :len(my)].T
    # recovery: pruned tokens copy their most-attending retained token's row
    for b in range(B):
        prune = sel[b]["prune"]
        out[b][prune] = out[b][sel[b]["src"][prune]]
    return out, res


def kernel(x, Wq, Wk, Wv, Wo, bo):
    proj, _ = run_proj(np.asarray(x, np.float32), np.asarray(Wq, np.float32),
                       np.asarray(Wk, np.float32), np.asarray(Wv, np.float32))
    attn, _ = run_map(proj)
    sel = host_select(attn)
    out, _ = run_attnr(proj, sel, np.asarray(Wo, np.float32),
                       np.asarray(bo, np.float32))
    return out


# revision 40
# speedup vs baseline: 2.1864x; 1.0442x over previous
"""Trainium2 Bass kernel for nn_MemoryEfficientCrossAttention (WPR-pruned attention).

Self-contained: hardcodes shapes/sharding. The harness calls kernel(**inputs).

Pipeline (3 SPMD launches on 8 NeuronCores + host selection):
  P1 proj:  core c: tokens [c*512,(c+1)*512) of flattened x -> qT,kT fp16 hi/lo
            pairs ([1152,512] each) via 3-product fp16 pair matmuls (x and W
            pairs are host-prepared fp16 casts), and vT fp16 via hi*hi.
  P2 map:   core (b,j): query rows j*512..+512 of batch b, all 16 heads ->
            attn row-chunk [512,2048] f32 = H * (mean over heads of softmax);
            the positive scale H is irrelevant to host selection. The 3 pair
            products are packed into 2 stacked matmuls per 128/88-row
            contraction tile pair (exact same product set, 2/3 the PE time).
            Exp+rowsum on ACT; weighted normalize-accumulate on DVE.
  Host sel: power iteration (f64) -> top-819 keep set; per-column argmax over
            retained rows -> recovery sources. Host gathers retained tokens.
  P3 attnr: core (b,j): retained-token quarter -> attention over retained
            keys, all fp16 single-pass (S^T, exp, PV with fused ones-column
            rowsums, /rowsum, @Wo + bo) -> finalT chunk, f32.
  Host: scatter retained rows, recovery copy final[prune] = final[src[prune]].
"""

import numpy as np

import concourse.bass as bass
import concourse.mybir as mybir
import concourse.tile as tile
from concourse.bass_utils import run_bass_kernel_spmd

F32 = mybir.dt.float32
F16 = mybir.dt.float16
AF = mybir.ActivationFunctionType

B, N, QD, H, D = 2, 2048, 1152, 16, 72
INNER = H * D
N_KEEP = 819
SCALE = np.float32(D ** -0.5)
P = 128
NC = 8
CHUNK = 512          # tokens per core in P1/P2
KT = QD // P         # 9 k-tiles of the 1152 contraction
NQP = 208            # padded retained tokens per quarter-core in P3
MP = 896             # padded retained-key count (819 -> 7 tiles of 128)

_CORE_IDS = list(range(NC))


def split_waits(nc, maxw=1):
    """This toolchain's walrus accepts only one sync-wait per instruction;
    move excess waits onto preceding same-engine EventSemaphore nops."""
    n_new = 0
    for f in nc.m.functions:
        for blk in f.blocks:
            out = []
            changed = False
            for inst in blk.instructions:
                si = inst.sync_info
                if si is not None and si.on_wait is not None and len(si.on_wait) > maxw:
                    waits = list(si.on_wait)
                    for w in waits[:-maxw]:
                        es = mybir.InstEventSemaphore(
                            name=f"Wsplit{n_new}", ins=[], outs=[])
                        es.engine = inst.engine
                        es.sync_info = mybir.SyncInfo(on_wait=[w], on_update=[])
                        out.append(es)
                        n_new += 1
                    si.on_wait = waits[-maxw:]
                    changed = True
                out.append(inst)
            if changed:
                blk.instructions = out
    return nc


# --------------------------------------------------------------------------
# P1: projections.  per core: xT_chunk [1152, 512] -> qT/kT fp16 pairs, vT fp16.
# --------------------------------------------------------------------------

def build_proj():
    nc = bass.Bass("TRN2", target_bir_lowering=False, debug=False, num_devices=NC)
    xT = nc.dram_tensor("xT", [QD, CHUNK], F32, kind="ExternalInput").ap()
    wq = nc.dram_tensor("wq", [QD, INNER], F32, kind="ExternalInput").ap()
    wk = nc.dram_tensor("wk", [QD, INNER], F32, kind="ExternalInput").ap()
    wv = nc.dram_tensor("wv", [QD, INNER], F32, kind="ExternalInput").ap()
    qhT = nc.dram_tensor("qhT", [INNER, CHUNK], F16, kind="ExternalOutput").ap()
    qlT = nc.dram_tensor("qlT", [INNER, CHUNK], F16, kind="ExternalOutput").ap()
    khT = nc.dram_tensor("khT", [INNER, CHUNK], F16, kind="ExternalOutput").ap()
    klT = nc.dram_tensor("klT", [INNER, CHUNK], F16, kind="ExternalOutput").ap()
    vT = nc.dram_tensor("vT", [INNER, CHUNK], F16, kind="ExternalOutput").ap()

    with tile.TileContext(nc) as tc:
        with tc.tile_pool(name="xp", bufs=1) as xp, \
             tc.tile_pool(name="sbp", bufs=2) as sbp, \
             tc.tile_pool(name="wp", bufs=3) as wp, \
             tc.tile_pool(name="op", bufs=6) as op, \
             tc.tile_pool(name="ps", bufs=4, space="PSUM") as ps:
            # x pairs resident; f32 staged through sub-band tiles.
            # x and wq sub-loads are interleaved so the first matmul only
            # gates on the first sub-band of each.
            xh = xp.tile([P, KT, CHUNK], F16)
            xl = xp.tile([P, KT, CHUNK], F16)

            def w_pairs(w_ap, lo=True, x_interleave=False):
                wh = wp.tile([P, KT, INNER], F16, tag="wh")
                wl = None
                if lo:
                    wl = wp.tile([P, KT, INNER], F16, tag="wl")
                for sub in range(3):
                    ksl = slice(sub * 3, sub * 3 + 3)
                    if x_interleave:
                        xt = sbp.tile([P, 3, CHUNK], F32, tag="xt")
                        nc.sync.dma_start(
                            xt[:],
                            xT.rearrange("(kc p) m -> p kc m", p=P)[:, ksl])
                        nc.scalar.copy(xh[:, ksl], xt[:])
                        nc.vector.tensor_sub(xl[:, ksl], xt[:], xh[:, ksl])
                    wt = sbp.tile([P, 3, INNER], F32, tag="wt")
                    nc.sync.dma_start(
                        wt[:], w_ap.rearrange("(kc p) m -> p kc m", p=P)[:, ksl])
                    nc.scalar.copy(wh[:, ksl], wt[:])
                    if lo:
                        nc.vector.tensor_sub(wl[:, ksl], wt[:], wh[:, ksl])
                return wh, wl

            # qT/kT = W^T @ xT  (out [1152(9 mt), 512]), emit fp16 hi/lo
            for w_ap, hiT, loT in ((wq, qhT, qlT), (wk, khT, klT)):
                wh, wl = w_pairs(w_ap, x_interleave=(w_ap is wq))
                for mt in range(KT):
                    pt = ps.tile([P, CHUNK], F32, tag="pt")
                    msl = slice(mt * P, (mt + 1) * P)
                    for kk in range(KT):
                        nc.tensor.matmul(pt[:], wh[:, kk, msl], xh[:, kk],
                                         start=(kk == 0), stop=False)
                        nc.tensor.matmul(pt[:], wh[:, kk, msl], xl[:, kk],
                                         start=False, stop=False)
                        nc.tensor.matmul(pt[:], wl[:, kk, msl], xh[:, kk],
                                         start=False, stop=(kk == KT - 1))
                    hi = op.tile([P, CHUNK], F16, tag="hi")
                    lo = op.tile([P, CHUNK], F16, tag="lo")
                    nc.scalar.copy(hi[:], pt[:])
                    nc.vector.tensor_sub(lo[:], pt[:], hi[:])
                    nc.sync.dma_start(hiT[mt * P:(mt + 1) * P, :], hi[:])
                    nc.sync.dma_start(loT[mt * P:(mt + 1) * P, :], lo[:])

            # vT = Wv^T @ xT, single fp16 product (output precision only)
            wvh, _ = w_pairs(wv, lo=False)
            for mt in range(KT):
                pv = ps.tile([P, CHUNK], F32, tag="pt")
                msl = slice(mt * P, (mt + 1) * P)
                for kk in range(KT):
                    nc.tensor.matmul(pv[:], wvh[:, kk, msl], xh[:, kk],
                                     start=(kk == 0), stop=(kk == KT - 1))
                vv = op.tile([P, CHUNK], F16, tag="hi")
                nc.scalar.copy(vv[:], pv[:])
                nc.sync.dma_start(vT[mt * P:(mt + 1) * P, :], vv[:])
    return split_waits(nc)


def run_proj(x, Wq, Wk, Wv, trace=False):
    """x [B,N,QD] f32 -> qhT,qlT,khT,klT,vT [B][INNER,N] fp16"""
    xf = np.ascontiguousarray(x.reshape(B * N, QD).T)  # [QD, 4096]
    in_maps = []
    for c in range(NC):
        in_maps.append({
            "xT": np.ascontiguousarray(xf[:, c * CHUNK:(c + 1) * CHUNK]),
            "wq": Wq, "wk": Wk, "wv": Wv,
        })
    res = run_bass_kernel_spmd(build_proj(), in_maps, core_ids=_CORE_IDS, trace=trace)
    outs = {}
    for name in ("qhT", "qlT", "khT", "klT", "vT"):
        full = np.concatenate([res.results[c][name] for c in range(NC)], axis=1)
        outs[name] = [full[:, b * N:(b + 1) * N] for b in range(B)]
    return outs, res


# --------------------------------------------------------------------------
# P2: attention map.  per core (b, j): query rows [j*512,(j+1)*512) of batch b
# -> attn row-chunk [512, 2048] f32 = mean over heads of row-softmax.
# The 3 fp16-pair products (qh.kh + qh.kl + ql.kh) are packed into 2 stacked
# contraction tiles of 128 and 88 rows:
#   tile A: rows [qh(0:72); qh(0:56)] x [kh(0:72); kl(0:56)]
#   tile B: rows [qh(56:72); ql(0:72)] x [kl(56:72); kh(0:72)]
# --------------------------------------------------------------------------

PB = 216 - P  # 88 rows in the second stacked tile


def build_map():
    nc = bass.Bass("TRN2", target_bir_lowering=False, debug=False, num_devices=NC)
    qh = nc.dram_tensor("qh", [INNER, CHUNK], F16, kind="ExternalInput").ap()
    ql = nc.dram_tensor("ql", [INNER, CHUNK], F16, kind="ExternalInput").ap()
    kh = nc.dram_tensor("kh", [INNER, N], F16, kind="ExternalInput").ap()
    kl = nc.dram_tensor("kl", [INNER, N], F16, kind="ExternalInput").ap()
    attn = nc.dram_tensor("attn", [CHUNK, N], F32, kind="ExternalOutput").ap()

    NSEG = N // 512  # 4 column segments per row
    qh_r = qh.rearrange("(h d) m -> d h m", d=D)
    ql_r = ql.rearrange("(h d) m -> d h m", d=D)
    kh_r = kh.rearrange("(h d) m -> d h m", d=D)
    kl_r = kl.rearrange("(h d) m -> d h m", d=D)
    with tile.TileContext(nc) as tc:
        with tc.tile_pool(name="kp", bufs=1) as kp, \
             tc.tile_pool(name="qp", bufs=2) as qp, \
             tc.tile_pool(name="ep", bufs=3) as ep, \
             tc.tile_pool(name="apA", bufs=3) as apA, \
             tc.tile_pool(name="sp", bufs=8) as sp, \
             tc.tile_pool(name="ps", bufs=2, space="PSUM") as ps:
            # resident stacked k tiles, one pair per head so a head's matmuls
            # only gate on their own small loads; q tiles per 128-query tile,
            # interleaved into the load order so the DMA queue serves the
            # compute front in consumption order.
            ksA = [None] * H
            ksB = [None] * H
            qsA = [None] * 4
            qsB = [None] * 4
            def load_k(h):
                hs = slice(h, h + 1)
                kA = kp.tile([P, 1, N], F16, name=f"ksA{h}")
                kB = kp.tile([PB, 1, N], F16, name=f"ksB{h}")
                nc.sync.dma_start(kA[0:D], kh_r[:, hs])
                nc.sync.dma_start(kA[D:P], kl_r[0:P - D, hs])
                nc.sync.dma_start(kB[0:D - (P - D)], kl_r[P - D:D, hs])
                nc.sync.dma_start(kB[D - (P - D):PB], kh_r[:, hs])
                ksA[h] = kA
                ksB[h] = kB
            def load_q(nqt):
                msl = slice(nqt * P, (nqt + 1) * P)
                qA = qp.tile([P, H, P], F16, tag="qsA")
                qB = qp.tile([PB, H, P], F16, tag="qsB")
                nc.sync.dma_start(qA[0:D], qh_r[:, :, msl])
                nc.sync.dma_start(qA[D:P], qh_r[0:P - D, :, msl])
                nc.sync.dma_start(qB[0:D - (P - D)], qh_r[P - D:D, :, msl])
                nc.sync.dma_start(qB[D - (P - D):PB], ql_r[:, :, msl])
                qsA[nqt] = qA
                qsB[nqt] = qB
            load_q(0)
            for h in range(4):
                load_k(h)
            load_q(1)
            for h in range(4, H):
                load_k(h)
            for nqt in range(CHUNK // P):
                qsl = slice(nqt * P, (nqt + 1) * P)
                accA = apA.tile([P, N], F32, tag="accA")
                for h in range(H):
                    et = ep.tile([P, N], F32, tag="et")
                    rs = sp.tile([P, 1], F32, tag="rs")
                    w = sp.tile([P, 1], F32, tag="w")
                    pt4 = ps.tile([P, N], F32, tag="pt4")
                    for ms in range(NSEG):
                        seg = slice(ms * 512, (ms + 1) * 512)
                        nc.tensor.matmul(pt4[:, seg], qsA[nqt][:, h],
                                         ksA[h][:, 0, seg],
                                         start=True, stop=False)
                        nc.tensor.matmul(pt4[:, seg], qsB[nqt][:, h],
                                         ksB[h][:, 0, seg],
                                         start=False, stop=True)
                    # attn output is H * mean-map (sum of per-head
                    # softmaxes); host selection is scale-invariant.
                    nc.scalar.activation(et[:], pt4[:], AF.Exp,
                                         scale=float(SCALE), accum_out=rs[:])
                    nc.vector.reciprocal(w[:], rs[:])
                    if h == 0:
                        nc.vector.tensor_scalar(
                            accA[:], et[:], w[:], scalar2=None,
                            op0=mybir.AluOpType.mult)
                    else:
                        nc.vector.scalar_tensor_tensor(
                            accA[:], et[:], w[:], accA[:],
                            op0=mybir.AluOpType.mult,
                            op1=mybir.AluOpType.add)
                nc.sync.dma_start(attn[qsl, :], accA[:])
                if nqt + 2 < CHUNK // P:
                    load_q(nqt + 2)
    return split_waits(nc)


def run_map(proj, trace=False):
    in_maps = []
    for c in range(NC):
        b, j = divmod(c, 4)
        sl = slice(j * CHUNK, (j + 1) * CHUNK)
        in_maps.append({
            "qh": np.ascontiguousarray(proj["qhT"][b][:, sl]),
            "ql": np.ascontiguousarray(proj["qlT"][b][:, sl]),
            "kh": proj["khT"][b], "kl": proj["klT"][b],
        })
    res = run_bass_kernel_spmd(build_map(), in_maps, core_ids=_CORE_IDS, trace=trace)
    attn = [np.concatenate([res.results[b * 4 + j]["attn"] for j in range(4)], axis=0)
            for b in range(B)]
    return attn, res


# --------------------------------------------------------------------------
# Host selection: 5-step power iteration (f64), strict top-819 by importance,
# per-column argmax over retained rows for recovery sources.
# --------------------------------------------------------------------------

def host_select(attn):
    sel = []
    for b in range(B):
        A = attn[b]
        dist = np.full((1, N), 1.0 / N, np.float64)
        A64 = A.astype(np.float64)
        for _ in range(5):
            dist = dist @ A64
        imp = dist[0]
        order = np.argsort(-imp, kind="stable")
        keep = np.sort(order[:N_KEEP])
        prune = np.sort(order[N_KEEP:])
        src = keep[A[keep].argmax(axis=0)]  # [N] most-attending retained row
        sel.append({"keep": keep, "prune": prune, "src": src})
    return sel


# --------------------------------------------------------------------------
# P3: retained attention + output projection, all fp16 single-pass.
# per core (b, j): ~205 retained tokens (host-gathered q columns, padded to
# NQP) -> finT [1152, NQP] = (softmax(qk) @ v / rowsum) @ Wo + bo, transposed.
# Padded keys have k=0 (exp->1) but v and the ones-column are 0, so they
# contribute nothing to PV or the rowsums; no masking needed.
# --------------------------------------------------------------------------

def build_attnr():
    nc = bass.Bass("TRN2", target_bir_lowering=False, debug=False, num_devices=NC)
    qs = nc.dram_tensor("qs", [INNER, NQP], F16, kind="ExternalInput").ap()
    ks = nc.dram_tensor("ks", [INNER, MP], F16, kind="ExternalInput").ap()
    v97 = nc.dram_tensor("v97", [P, MP // P, H, 97], F16,
                         kind="ExternalInput").ap()  # v cols 0..71, ones col 96
    wo = nc.dram_tensor("wo", [INNER, INNER], F16, kind="ExternalInput").ap()
    boc = nc.dram_tensor("boc", [P, KT], F32, kind="ExternalInput").ap()
    finT = nc.dram_tensor("finT", [INNER, NQP], F32, kind="ExternalOutput").ap()

    NT = MP // P
    with tile.TileContext(nc) as tc:
        with tc.tile_pool(name="kp", bufs=1) as kp, \
             tc.tile_pool(name="ep", bufs=12) as ep, \
             tc.tile_pool(name="np_", bufs=12) as np_, \
             tc.tile_pool(name="op", bufs=1) as op, \
             tc.tile_pool(name="cp", bufs=1) as cp, \
             tc.tile_pool(name="fp", bufs=4) as fp, \
             tc.tile_pool(name="psS", bufs=3, space="PSUM") as psS, \
             tc.tile_pool(name="psO", bufs=2, space="PSUM") as psO, \
             tc.tile_pool(name="psB", bufs=1, space="PSUM") as psB, \
             tc.tile_pool(name="psF", bufs=2, space="PSUM") as psF:
            ks_r = ks.rearrange("(h d) m -> d h m", d=D)
            qst = kp.tile([D, H, NQP], F16)
            nc.sync.dma_start(qst[:], qs.rearrange("(h d) m -> d h m", d=D))
            kst = []
            KG = 4
            for hg in range(H // KG):
                kt_h = kp.tile([D, KG, MP], F16, name=f"kst{hg}")
                nc.sync.dma_start(kt_h[:], ks_r[:, hg * KG:(hg + 1) * KG])
                kst.append(kt_h)
            vr = []
            for mc in range(NT):
                vt = kp.tile([P, H, 97], F16, name=f"vr{mc}")
                nc.sync.dma_start(vt[:], v97[:, mc])
                vr.append(vt)
            wot = kp.tile([P, KT, INNER], F16)
            nc.sync.dma_start(wot[:], wo.rearrange("(kt p) m -> p kt m", p=P))
            bo_sb = cp.tile([P, KT], F32)
            nc.sync.dma_start(bo_sb[:], boc[:])
            ones72 = cp.tile([1, D], F16)
            nc.vector.memset(ones72[:], 1.0)

            ON = op.tile([P, KT, NQP], F16)  # normalized PV output, hd-stacked
            for h in range(H):
                # S^T logits in mc-pair PSUM tiles (2*208*4B fits one bank)
                et = ep.tile([P, NT, NQP], F16, tag="et")
                for mp0 in range(0, NT, 2):
                    w2 = min(2, NT - mp0)
                    pss = psS.tile([P, 2, NQP], F32, tag="pss")
                    for mc in range(mp0, mp0 + w2):
                        nc.tensor.matmul(pss[:, mc - mp0],
                                         kst[h // 4][:, h % 4,
                                                     mc * P:(mc + 1) * P],
                                         qst[:, h], start=True, stop=True)
                    nc.scalar.activation(et[:, mp0:mp0 + w2], pss[:, 0:w2],
                                         AF.Exp, scale=float(SCALE))
                po = psO.tile([97, NQP], F32, tag="po")
                for mc in range(NT):
                    nc.tensor.matmul(po[:], vr[mc][:, h], et[:, mc],
                                     start=(mc == 0), stop=(mc == NT - 1))
                # normalize: rows 0..71 / row 96 (the ones-column sums)
                inv = np_.tile([1, NQP], F16, tag="inv")
                with nc.allow_low_precision(reason="fp16 1/rowsum is plenty "
                                            "for the 2e-2 output tolerance"):
                    nc.vector.reciprocal(inv[:], po[96:97, :])
                pb = psB.tile([D, NQP], F32, tag="pb")
                nc.tensor.matmul(pb[:], ones72[:], inv[:], start=True, stop=True)
                binv = np_.tile([D, NQP], F16, tag="binv")
                nc.vector.tensor_scalar(binv[:], pb[:], 1.0, scalar2=None,
                                        op0=mybir.AluOpType.mult)
                onv = np_.tile([D, NQP], F16, tag="onv")
                nc.vector.tensor_mul(onv[:], po[0:D, :], binv[:])
                g0 = h * D
                t0, p0 = divmod(g0, P)
                n0 = min(D, P - p0)
                nc.sync.dma_start(ON[p0:p0 + n0, t0, :], onv[0:n0, :])
                if n0 < D:
                    nc.sync.dma_start(ON[0:D - n0, t0 + 1, :], onv[n0:D, :])

            # final^T = Wo^T @ ON + bo
            for mt in range(KT):
                pf = psF.tile([P, NQP], F32, tag="pf")
                for kt in range(KT):
                    nc.tensor.matmul(pf[:], wot[:, kt, mt * P:(mt + 1) * P],
                                     ON[:, kt, :],
                                     start=(kt == 0), stop=(kt == KT - 1))
                foa = fp.tile([P, NQP], F32, tag="foa")
                nc.vector.tensor_scalar(foa[:], pf[:], bo_sb[:, mt:mt + 1],
                                        scalar2=None, op0=mybir.AluOpType.add)
                nc.sync.dma_start(finT[mt * P:(mt + 1) * P, :], foa[:])
    return split_waits(nc)


def run_attnr(proj, sel, Wo, bo, trace=False):
    in_maps = []
    meta = []
    boc = np.ascontiguousarray(bo.reshape(KT, P).T.astype(np.float32))
    wo16 = np.ascontiguousarray(Wo.astype(np.float16))
    for c in range(NC):
        b, j = divmod(c, 4)
        idx = sel[b]["keep"]
        bounds = np.linspace(0, len(idx), 5).astype(int)
        my = idx[bounds[j]:bounds[j + 1]]
        meta.append(my)
        qg = np.zeros((INNER, NQP), np.float16)
        qg[:, :len(my)] = proj["qhT"][b][:, my]
        kg = np.zeros((INNER, MP), np.float16)
        kg[:, :len(idx)] = proj["khT"][b][:, idx]
        vsel = np.zeros((MP, INNER), np.float16)
        vsel[:len(idx)] = proj["vT"][b][:, idx].T
        v97 = np.zeros((P, MP // P, H, 97), np.float16)
        v97[..., :D] = (vsel.reshape(MP // P, P, H, D)).transpose(1, 0, 2, 3)
        ones = np.zeros(MP, np.float16)
        ones[:len(idx)] = 1.0
        v97[..., 96] = ones.reshape(MP // P, P).T[:, :, None]
        in_maps.append({
            "qs": qg, "ks": kg,
            "v97": np.ascontiguousarray(v97),
            "wo": wo16, "boc": boc,
        })
    res = run_bass_kernel_spmd(build_attnr(), in_maps, core_ids=_CORE_IDS, trace=trace)
    out = np.zeros((B, N, INNER), np.float32)
    for c in range(NC):
        b = c // 4
        my = meta[c]
        out[b][my] = res.results[c]["finT"][:, :len(my)].T
    # recovery: pruned tokens copy their most-attending retained token's row
    for b in range(B):
        prune = sel[b]["prune"]
        out[b][prune] = out[b][sel[b]["src"][prune]]
    return out, res


def kernel(x, Wq, Wk, Wv, Wo, bo):
    proj, _ = run_proj(np.asarray(x, np.float32), np.asarray(Wq, np.float32),
                       np.asarray(Wk, np.float32), np.asarray(Wv, np.float32))
    attn, _ = run_map(proj)
    sel = host_select(attn)
    out, _ = run_attnr(proj, sel, np.asarray(Wo, np.float32),
                       np.asarray(bo, np.float32))
    return out
